# revision 1
# baseline (speedup 1.0000x reference)
"""Masked video loss kernel for TRN2 (8 NeuronCores, SPMD).

Algorithmic structure exploited: the decoder input feat_3d is spatially
constant (broadcast of per-frame features over H=W=64), so the three
SAME-padded 3x3x3 convs produce at most 7x7 distinct values per (b,c,t)
(spatial boundary classes at distance 0,1,2,interior,-3,-2,-1 from each
edge). We evaluate the decoder on a 7x7 spatial grid (exact, not an
approximation) and fold the masked MSE through per-class statistics:

  sum_masked (r - o)^2 = r^2 * cnt - 2 r * s1 + s2
     cnt = sum_masked 1, s1 = sum_masked o, s2 = sum_masked o^2

per class. s2 needs no class split (appears fully summed).

Sharding: core = 2*b + th. Each core runs encoder+decoder for batch b
over full T=16 (identical SPMD program), and computes mask stats only
for its t-half (host zeroes the other half of its mask copy).
Host does the final ~10k-flop assembly of the gathered class tensors.
"""

import sys

sys.path.insert(0, "/opt/trn_rl_repo")

from contextlib import ExitStack  # noqa: E402

import numpy as np  # noqa: E402

import concourse.bacc as bacc  # noqa: E402
import concourse.bass as bass  # noqa: E402
import concourse.mybir as mybir  # noqa: E402
import concourse.tile as tile  # noqa: E402
from concourse import bass_utils  # noqa: E402

B, T, C, H, W = 4, 16, 3, 64, 64
D = 256
X = C * H * W  # 12288
HW = H * W  # 4096
NCORES = 8

F32 = mybir.dt.float32
F32R = mybir.dt.float32r
BF16 = mybir.dt.bfloat16
U8 = mybir.dt.uint8

# spatial boundary classes after 3 stacked 3x3 SAME convs
CLS_BOUNDS = [0, 1, 2, 3, H - 3, H - 2, H - 1, H]  # 7 classes
NCLS = 7
# padded 7x7 grid: 9x9 with 1-pixel zero ring; t padded 16 -> 18
GI, GJ = NCLS, NCLS
PI, PJ = GI + 2, GJ + 2  # 9, 9
PT = T + 2  # 18
GJ8 = GJ  # no junk col needed for bf16 matmuls
NVOX_H = 8 * GI * GJ8  # 392 voxels per t-half (8 frames x 7 x 7)


def _emit(nc, a_in, a_out):
    """Emit the per-core program. a_in/a_out: dicts name -> bass.AP."""
    ctx = ExitStack()
    tc = tile.TileContext(nc)
    with tc, ctx:
        io = ctx.enter_context(tc.tile_pool(name="io", bufs=1))
        wenc_pool = ctx.enter_context(tc.tile_pool(name="wenc", bufs=3))
        work = ctx.enter_context(tc.tile_pool(name="work", bufs=1))
        ps_enc = ctx.enter_context(tc.tile_pool(name="ps_enc", bufs=1, space="PSUM"))
        ps = ctx.enter_context(tc.tile_pool(name="ps", bufs=4, space="PSUM"))

        # ---------------- input loads (all host-prepermuted, contiguous) ----
        # big1 columns: [0:1536) obsT | [1536:2048) keepT (1-mask, f32)
        big1 = io.tile([128, 2048], F32)
        nc.sync.dma_start(big1[:], a_in["big1"])
        obsT_sb = big1[:, 0:1536]
        keep = big1[:, 1536:2048]
        # big2 columns: [0:1536) obs_st | [1536:2048) mask_st (f32)
        big2 = io.tile([128, 2048], F32)
        nc.sync.dma_start(big2[:], a_in["big2"])
        O_sb = big2[:, 0:1536]
        mf_st = big2[:, 1536:2048]
        # conv weights: one bf16 tensor [128, 6912+1728+81]
        wc = io.tile([128, 27 * 2 * 128 + 27 * 64 + 27 * 3], BF16)
        nc.sync.dma_start(wc[:], a_in["wconv"])
        w1T_sb = wc[:, 0 : 27 * 2 * 128]
        w2T_sb = wc[:, 27 * 2 * 128 : 27 * 2 * 128 + 27 * 64]
        w3T_sb = wc[0:64, 27 * 2 * 128 + 27 * 64 : 27 * 2 * 128 + 27 * 64 + 27 * 3]
        # consts [128, 35]: [0:2) benc | [2] b1 | [3] b2 (rows<64) | [4] b3 (rows<3)
        #                   [5:19) rhT | [19:35) eye16 (rows<16)
        cons = io.tile([128, 35], F32)
        nc.sync.dma_start(cons[:], a_in["consts"])
        benc_sb = cons[:, 0:2]
        b1_sb = cons[:, 2:3]
        b2_sb = cons[0:64, 3:4]
        b3_sb = cons[0:3, 4:5]
        rhT_sb = cons[:, 5:19]
        eye_sb = cons[0:16, 19:35]

        outv = work.tile([128, 225], F32, tag="outv")
        nc.gpsimd.memset(outv[:], 0.0)

        # ---------------- PE warm-up (HAM clock-gate) during input DMAs ----
        ps_warm = ctx.enter_context(tc.tile_pool(name="ps_warm", bufs=1, space="PSUM"))
        warm_ps = ps_warm.tile([2, 35], F32)
        for i in range(30):
            nc.tensor.matmul(
                warm_ps[:], cons[:, 0:2], cons[:, 0:35], start=(i == 0), stop=(i == 29)
            )
        nc.vector.tensor_copy(outv[96:98, 0:35], warm_ps[:])

        # ---------------- encoder input masking (DVE) ----------------
        xt = work.tile([128, 96 * T], BF16, tag="xt")
        for c in range(C):
            sl = slice(c * 32 * T, (c + 1) * 32 * T)
            nc.vector.tensor_mul(xt[:, sl], obsT_sb[:, sl], keep)

        # ---------------- encoder matmuls: featsT [16, 256] ----------------
        featsT_ps = ps_enc.tile([16, D], F32)
        # stream W_encT in 4 chunks of [128, 24*256]
        for g in range(4):
            wk = wenc_pool.tile([128, 24 * D], BF16, tag="wk")
            nc.sync.dma_start(wk[:], a_in["wencT"][g])
            for r in range(24):
                ki = g * 24 + r
                nc.tensor.matmul(
                    featsT_ps[:],
                    xt[:, ki * T : (ki + 1) * T],
                    wk[:, r * D : (r + 1) * D],
                    start=(ki == 0),
                    stop=(ki == 95),
                )
        featsT_sb = work.tile([16, D], F32, tag="ftsb")
        nc.vector.tensor_copy(featsT_sb[:], featsT_ps[:])
        # transpose to feats [128=(d%128), kd:2, t:16] and add b_enc
        feats_sb = work.tile([128, 2 * T], F32, tag="feats")
        for kd in range(2):
            tr_ps = ps.tile([128, 16], F32, tag="cv")
            nc.tensor.transpose(
                tr_ps[:], featsT_sb[:, kd * 128 : (kd + 1) * 128], eye_sb
            )
            nc.scalar.activation(
                feats_sb[:, kd * T : (kd + 1) * T],
                tr_ps[:],
                mybir.ActivationFunctionType.Identity,
                bias=benc_sb[:, kd : kd + 1],
            )

        # ---------------- broadcast into padded conv input ----------------
        # xpad1 [128, kd:2, t:18, i:9, j:9]
        xpad1 = work.tile([128, 2 * PT * PI * PJ], BF16, tag="xpad1")
        nc.gpsimd.memset(xpad1[:], 0.0)
        v1x = xpad1[:].rearrange("p (kd t i j) -> p kd t i j", kd=2, t=PT, i=PI, j=PJ)
        for kd in range(2):
            src = (
                feats_sb[:, kd * T : (kd + 1) * T]
                .unsqueeze(2)
                .unsqueeze(3)
                .broadcast_to([128, T, GI, GJ])
            )
            nc.vector.tensor_copy(v1x[:, kd, 1 : T + 1, 1 : 1 + GI, 1 : 1 + GJ], src)

        # ---------------- conv1 (+relu) ----------------
        h1_sb = work.tile([128, T * GI * GJ8], F32, tag="h1")
        for hf in range(2):
            p1 = ps.tile([128, NVOX_H], F32, tag="cv")
            first = True
            for kt in range(3):
                for kh in range(3):
                    for kw in range(3):
                        tap = (kt * 3 + kh) * 3 + kw
                        for kd in range(2):
                            rhs = v1x[
                                :,
                                kd,
                                8 * hf + kt : 8 * hf + kt + 8,
                                kh : kh + GI,
                                kw : kw + GJ8,
                            ]
                            nc.tensor.matmul(
                                p1[:],
                                w1T_sb[
                                    :, (tap * 2 + kd) * 128 : (tap * 2 + kd + 1) * 128
                                ],
                                rhs,
                                start=first,
                                stop=(tap == 26 and kd == 1),
                            )
                            first = False
            nc.scalar.activation(
                h1_sb[:, hf * NVOX_H : (hf + 1) * NVOX_H],
                p1[:],
                mybir.ActivationFunctionType.Relu,
                bias=b1_sb[:, 0:1],
            )

        xpad2 = work.tile([128, PT * PI * PJ], BF16, tag="xpad2")
        nc.gpsimd.memset(xpad2[:], 0.0)
        v2x = xpad2[:].rearrange("p (t i j) -> p t i j", t=PT, i=PI, j=PJ)
        nc.vector.tensor_copy(
            v2x[:, 1 : T + 1, 1 : 1 + GI, 1 : 1 + GJ],
            h1_sb[:].rearrange("p (t i j) -> p t i j", t=T, i=GI, j=GJ8)[:, :, :, :GJ],
        )

        # ---------------- conv2 (+relu) ----------------
        h2_sb = work.tile([64, T * GI * GJ8], F32, tag="h2")
        for hf in range(2):
            p2 = ps.tile([64, NVOX_H], F32, tag="cv")
            for kt in range(3):
                for kh in range(3):
                    for kw in range(3):
                        tap = (kt * 3 + kh) * 3 + kw
                        rhs = v2x[
                            :, 8 * hf + kt : 8 * hf + kt + 8, kh : kh + GI, kw : kw + GJ8
                        ]
                        nc.tensor.matmul(
                            p2[:],
                            w2T_sb[:, tap * 64 : (tap + 1) * 64],
                            rhs,
                            start=(tap == 0),
                            stop=(tap == 26),
                        )
            nc.scalar.activation(
                h2_sb[:, hf * NVOX_H : (hf + 1) * NVOX_H],
                p2[:],
                mybir.ActivationFunctionType.Relu,
                bias=b2_sb[:, 0:1],
            )

        xpad3 = work.tile([64, PT * PI * PJ], BF16, tag="xpad3")
        nc.gpsimd.memset(xpad3[:], 0.0)
        v3x = xpad3[:].rearrange("p (t i j) -> p t i j", t=PT, i=PI, j=PJ)
        nc.vector.tensor_copy(
            v3x[:, 1 : T + 1, 1 : 1 + GI, 1 : 1 + GJ],
            h2_sb[:].rearrange("p (t i j) -> p t i j", t=T, i=GI, j=GJ8)[:, :, :, :GJ],
        )

        # ---------------- conv3 (+bias) -> recon classes ----------------
        recon_sb = work.tile([3, T * GI * GJ8], F32, tag="recon")
        for hf in range(2):
            p3 = ps.tile([3, NVOX_H], F32, tag="cv")
            for kt in range(3):
                for kh in range(3):
                    for kw in range(3):
                        tap = (kt * 3 + kh) * 3 + kw
                        rhs = v3x[
                            :, 8 * hf + kt : 8 * hf + kt + 8, kh : kh + GI, kw : kw + GJ8
                        ]
                        nc.tensor.matmul(
                            p3[:],
                            w3T_sb[:, tap * 3 : (tap + 1) * 3],
                            rhs,
                            start=(tap == 0),
                            stop=(tap == 26),
                        )
            nc.scalar.activation(
                recon_sb[:, hf * NVOX_H : (hf + 1) * NVOX_H],
                p3[:],
                mybir.ActivationFunctionType.Identity,
                bias=b3_sb[:, 0:1],
            )
        nc.sync.dma_start(a_out["recon_cls"], recon_sb[:])

        # ---------------- mask stats ----------------
        vO = O_sb.rearrange("p (tt c w) -> p tt c w", tt=8, c=C)
        vM = mf_st.rearrange("p (tt w) -> p tt w", tt=8)
        mo = work.tile([128, 8 * C * W], F32, tag="mo")
        vmo = mo[:].rearrange("p (tt c w) -> p tt c w", tt=8, c=C)
        for c in range(C):
            nc.vector.tensor_mul(vmo[:, :, c, :], vO[:, :, c, :], vM[:])
        mo2 = work.tile([128, 8 * C * W], F32, tag="mo2")
        nc.vector.tensor_mul(mo2[:], mo[:], O_sb)
        # s2 partial (sum over everything later on host)
        nc.vector.reduce_sum(outv[:, 224:225], mo2[:], axis=mybir.AxisListType.X)
        # w-class segmented reduce: U1 [128, (tt,c,j)], Uc [128, (tt,j)]
        U1 = work.tile([128, 8 * C * NCLS], F32, tag="U1")
        vU1 = U1[:].rearrange("p (tt c j) -> p tt c j", tt=8, c=C)
        Uc = work.tile([128, 8 * NCLS], F32, tag="Uc")
        vUc = Uc[:].rearrange("p (tt j) -> p tt j", tt=8)
        vmo4 = mo[:].rearrange("p (tt c w) -> p tt c w", tt=8, c=C)
        for j in range(NCLS):
            w0, w1_ = CLS_BOUNDS[j], CLS_BOUNDS[j + 1]
            nc.vector.reduce_sum(
                vU1[:, :, :, j], vmo4[:, :, :, w0:w1_], axis=mybir.AxisListType.X
            )
            nc.vector.reduce_sum(
                vUc[:, :, j], vM[:, :, w0:w1_], axis=mybir.AxisListType.X
            )
        # h-class reduce via PE: V1 [14, 168], Vc [14, 56]
        pv1 = ps.tile([14, 8 * C * NCLS], F32, tag="cv")
        nc.tensor.matmul(pv1[:], rhT_sb, U1[:], start=True, stop=True)
        nc.vector.tensor_copy(outv[0:14, 0:168], pv1[:])
        pvc = ps.tile([14, 8 * NCLS], F32, tag="cv")
        nc.tensor.matmul(pvc[:], rhT_sb, Uc[:], start=True, stop=True)
        nc.vector.tensor_copy(outv[0:14, 168:224], pvc[:])
        nc.sync.dma_start(a_out["outv"], outv[:])


_CACHE = {}


def _build():
    if "nc" in _CACHE:
        return _CACHE["nc"]
    nc = bacc.Bacc("TRN2", target_bir_lowering=False, debug=False)
    a_in = {}

    def din(name, shape, dt):
        a_in[name] = nc.dram_tensor(name, shape, dt, kind="ExternalInput").ap()

    din("big1", (128, 2048), F32)
    din("big2", (128, 2048), F32)
    din("wencT", (4, 128, 24 * D), BF16)
    din("wconv", (128, 27 * 2 * 128 + 27 * 64 + 27 * 3), BF16)
    din("consts", (128, 35), F32)
    a_out = {}
    for name, shape in [
        ("recon_cls", (3, T * GI * GJ8)),
        ("outv", (128, 225)),
    ]:
        a_out[name] = nc.dram_tensor(name, shape, F32, kind="ExternalOutput").ap()
    _emit(nc, a_in, a_out)
    nc.compile()
    _CACHE["nc"] = nc
    return nc


def make_in_maps(obs_strip, mask, W_enc, b_enc, w1, b1, w2, b2, w3, b3):
    import ml_dtypes

    bf16 = ml_dtypes.bfloat16
    obs_strip = np.ascontiguousarray(obs_strip, dtype=np.float32)
    mask_f = np.ascontiguousarray(mask).astype(np.float32)
    rh = np.zeros((7, 64), np.float32)
    for i in range(NCLS):
        rh[i, CLS_BOUNDS[i] : CLS_BOUNDS[i + 1]] = 1.0
    rhT = np.zeros((128, 14), np.float32)
    for u in range(2):
        rhT[u * 64 : (u + 1) * 64, u * 7 : (u + 1) * 7] = rh.T
    consts = np.zeros((128, 35), np.float32)
    consts[:, 0:2] = np.asarray(b_enc, np.float32).reshape(2, 128).T
    consts[:, 2] = np.asarray(b1, np.float32)
    consts[0:64, 3] = np.asarray(b2, np.float32)
    consts[0:3, 4] = np.asarray(b3, np.float32)
    consts[:, 5:19] = rhT
    consts[0:16, 19:35] = np.eye(16, dtype=np.float32)
    wconv = np.zeros((128, 27 * 2 * 128 + 27 * 64 + 27 * 3), bf16)
    wconv[:, 0 : 27 * 2 * 128] = (
        np.ascontiguousarray(w1)
        .transpose(2, 3, 4, 1, 0)
        .reshape(27, 2, 128, 128)
        .transpose(2, 0, 1, 3)
        .reshape(128, 27 * 2 * 128)
        .astype(bf16)
    )
    wconv[:, 27 * 2 * 128 : 27 * 2 * 128 + 27 * 64] = (
        np.ascontiguousarray(w2)
        .transpose(2, 3, 4, 1, 0)
        .reshape(27, 128, 64)
        .transpose(1, 0, 2)
        .reshape(128, 27 * 64)
        .astype(bf16)
    )
    wconv[0:64, 27 * 2 * 128 + 27 * 64 :] = (
        np.ascontiguousarray(w3)
        .transpose(2, 3, 4, 1, 0)
        .reshape(27, 64, 3)
        .transpose(1, 0, 2)
        .reshape(64, 27 * 3)
        .astype(bf16)
    )
    wencT = np.ascontiguousarray(
        np.asarray(W_enc, np.float32)
        .T.reshape(4, 24, 128, D)
        .transpose(0, 2, 1, 3)
        .reshape(4, 128, 24 * D)
    ).astype(bf16)
    shared = {"wencT": wencT, "wconv": wconv, "consts": consts}
    in_maps = []
    for core in range(NCORES):
        b, th = core // 2, core % 2
        mask_st = mask_f[b].copy()
        if th == 0:
            mask_st[8:] = 0.0
        else:
            mask_st[:8] = 0.0
        big1 = np.empty((128, 2048), np.float32)
        big1[:, 0:1536] = (
            obs_strip[b].reshape(T, 96, 128).transpose(2, 1, 0).reshape(128, 96 * T)
        )
        big1[:, 1536:2048] = 1.0 - mask_f[b].reshape(T, 32, 128).transpose(
            2, 1, 0
        ).reshape(128, 32 * T)
        big2 = np.empty((128, 2048), np.float32)
        big2[:, 0:1536] = (
            obs_strip[b]
            .reshape(8, 2, C, H, W)
            .transpose(1, 3, 0, 2, 4)
            .reshape(128, 8 * C * W)
        )
        big2[:, 1536:2048] = mask_st.reshape(8, 2, H, W).transpose(1, 2, 0, 3).reshape(
            128, 8 * W
        )
        in_maps.append({"big1": big1, "big2": big2, **shared})
    return in_maps


def assemble(results):
    total_sq = 0.0
    total_cnt = 0.0
    for core in range(NCORES):
        r = results[core]
        recon = r["recon_cls"].astype(np.float64).reshape(3, T, GI, GJ8)[..., :GJ]
        outv = r["outv"].astype(np.float64)
        v1 = outv[0:14, 0:168].reshape(2, NCLS, 8, C, NCLS)  # [u,i,tt,c,j]
        vc = outv[0:14, 168:224].reshape(2, NCLS, 8, NCLS)  # [u,i,tt,j]
        s2 = float(outv[:, 224].sum())
        s1 = np.zeros((T, C, NCLS, NCLS))
        cnt = np.zeros((T, NCLS, NCLS))
        for u in range(2):
            s1[u::2] = v1[u].transpose(1, 2, 0, 3)  # [tt,c,i,j]
            cnt[u::2] = vc[u].transpose(1, 0, 2)  # [tt,i,j]
        rt = recon.transpose(1, 0, 2, 3)  # [t,c,i,j]
        total_sq += float((rt * rt * cnt[:, None]).sum() - 2.0 * (rt * s1).sum() + s2)
        total_cnt += float(cnt.sum())
    loss = total_sq / max(total_cnt * C, 1.0)
    return np.float32(loss)


def kernel(**inputs):
    nc = _build()
    in_maps = make_in_maps(**inputs)
    res = bass_utils.run_bass_kernel_spmd(nc, in_maps, core_ids=list(range(NCORES)))
    _CACHE["last_res"] = res
    return assemble(res.results)


if __name__ == "__main__":
    pass



# revision 10
# speedup vs baseline: 2.8473x; 2.8473x over previous
"""Masked video loss kernel for TRN2 (8 NeuronCores, SPMD).

Algorithmic structure exploited:
- The decoder input feat_3d is spatially constant (broadcast of per-frame
  features over H=W=64), so conv1 collapses to a per-frame linear map with
  9 edge-variant weight sums (W1eff), evaluated directly on a 5x5 class
  grid. conv2 runs as a true 3x3x3 conv on the (padded) 5-grid; its output
  is expanded to the 7-grid on which conv3 produces the 7x7 recon classes.
  All exact (class algebra), not approximations.
- Masked MSE folds through per-class stats: sum (r-o)^2 = r^2 cnt - 2 r s1
  + s2 per (t, 7x7 class); s1/cnt come from one 0/1-matrix PE matmul plus
  segmented DVE reduces.

Sharding: core = 2*b + th. Each core handles batch b and an 11-frame
t-window starting at s = 5*th (host shifts the data, so the program is
SPMD-uniform); decoder outputs are valid for the core's 8-frame t-half,
and mask stats are host-zeroed outside that half.

Precision: W_enc / w1eff / w2 in fp8-e4m3 (stationary operands), obs /
activations bf16, accumulation fp32. Measured end-to-end loss rel err
~5e-3 (gate 2e-2).
"""

import sys

sys.path.insert(0, "/opt/trn_rl_repo")

from contextlib import ExitStack  # noqa: E402

import numpy as np  # noqa: E402

import concourse.bacc as bacc  # noqa: E402
import concourse.mybir as mybir  # noqa: E402
import concourse.tile as tile  # noqa: E402
from concourse import bass_utils  # noqa: E402

B, T, C, H, W = 4, 16, 3, 64, 64
D = 256
NCORES = 8

F32 = mybir.dt.float32
BF16 = mybir.dt.bfloat16
F8 = mybir.dt.float8e4

WIN = 11          # feats/conv t-window frames per core
WP = WIN + 2      # padded window
M35 = [0, 1, 1, 1, 2]          # 5-grid pos -> 3-class variant
M57 = [0, 1, 2, 2, 2, 3, 4]    # 7-grid pos -> 5-grid src index
# expansion groups (dst0, dstlen, src0, srclen) along one axis for 5->7
G57 = [(0, 2, 0, 2), (2, 3, 2, 1), (5, 2, 3, 2)]
# h2-row groups for segmented stats reduction (h = 2*h2 + hpar)
H2G = [(0, 1), (1, 2), (2, 30), (30, 31), (31, 32)]
NG = len(H2G)
WCLS_BOUNDS = [0, 1, 2, 3, 61, 62, 63, 64]

NU1 = 3 * NG * WIN       # 165
NUC = NG * WIN           # 55
NSTAT = NU1 + NUC + 1    # 221


def _emit(nc, a_in, a_out):
    ctx = ExitStack()
    tc = tile.TileContext(nc)
    with tc, ctx:
        io = ctx.enter_context(tc.tile_pool(name="io", bufs=1))
        wkp = ctx.enter_context(tc.tile_pool(name="wkp", bufs=2))
        work = ctx.enter_context(tc.tile_pool(name="work", bufs=1))
        ps = ctx.enter_context(tc.tile_pool(name="ps", bufs=1, space="PSUM"))

        # ---------- early memsets (Pool; no deps) ----------
        fpad = work.tile([128, 2 * WP], BF16, tag="fpad")
        nc.gpsimd.memset(fpad[:], 0.0)
        h1p = work.tile([128, WP * 7 * 7], BF16, tag="h1p")
        nc.gpsimd.memset(h1p[:], 0.0)
        h2p = work.tile([65, WP * 9 * 9 + 2], BF16, tag="h2p")
        nc.gpsimd.memset(h2p[0:64, :], 0.0)
        nc.gpsimd.memset(h2p[64:65, :], 1.0)

        # ---------- input DMAs (serialized by the DMA engine) ----------
        cons = io.tile([128, 8], F32)
        nc.sync.dma_start(cons[:], a_in["consts"])
        om = io.tile([128, 1760], BF16)
        nc.sync.dma_start(om[:], a_in["obsmask"])
        obs = om[:, 0:1056]
        keep = om[:, 1056:1408]
        mst = om[:, 1408:1760]

        # ---------- PE warm-up (p-state ramp) ----------
        warm_ps = ps.tile([2, 512], F32, tag="warm")
        for i in range(24):
            nc.tensor.matmul(
                warm_ps[:, 0:8], cons[:, 0:2], cons[:, 0:8],
                start=(i == 0), stop=(i == 23),
            )
        for i in range(10):
            nc.tensor.matmul(
                warm_ps[:], om[:, 0:2], om[:, 0:512],
                start=(i == 0), stop=(i == 9),
            )

        # ---------- encoder input masking ----------
        xt = work.tile([128, 3 * 32 * WIN], BF16, tag="xt")
        for c in range(C):
            sl = slice(c * 32 * WIN, (c + 1) * 32 * WIN)
            nc.vector.tensor_mul(xt[:, sl], obs[:, sl], keep)

        # ---------- encoder matmuls: feats [d, tau] in two d-halves ----
        feats0 = ps.tile([128, WIN], F32, tag="feats0")
        feats1 = ps.tile([128, WIN], F32, tag="feats1")
        fps = [feats0, feats1]
        for g in range(4):
            wk = wkp.tile([128, 24 * D], F8, tag="wk")
            nc.sync.dma_start(wk[:], a_in["wencT"][g])
            for r in range(24):
                ki = g * 24 + r
                for u in range(2):
                    nc.tensor.matmul(
                        fps[u][:],
                        wk[:, r * D + u * 128: r * D + (u + 1) * 128],
                        xt[:, ki * WIN: (ki + 1) * WIN],
                        start=(ki == 0),
                        stop=(ki == 95),
                    )
            if g < 3:
                for i in range(9):
                    nc.tensor.matmul(
                        warm_ps[:], om[:, 0:2], om[:, 0:512],
                        start=(i == 0), stop=(i == 8),
                    )

        # conv weights after wencT (conv chain is later anyway)
        wc1 = io.tile([128, 54 * 128], F8)
        nc.sync.dma_start(wc1[:], a_in["wc1"])
        wc2 = io.tile([128, 27 * 64], F8)
        nc.sync.dma_start(wc2[:], a_in["wc2"])
        wc3 = io.tile([128, 98], BF16)
        nc.sync.dma_start(wc3[:], a_in["wc3"])

        # feats + b_enc -> fpad interior (bf16)
        for u in range(2):
            nc.scalar.activation(
                fpad[:, u * WP + 1: u * WP + 1 + WIN],
                fps[u][:],
                mybir.ActivationFunctionType.Identity,
                bias=cons[:, u: u + 1],
            )

        # ---------- mask stats (DVE; overlapped with DMA/encoder) ------
        mo = work.tile([128, 1056], BF16, tag="mo")
        for c in range(C):
            sl = slice(c * 32 * WIN, (c + 1) * 32 * WIN)
            nc.vector.tensor_mul(mo[:, sl], obs[:, sl], mst)
        mo2 = work.tile([128, 1056], BF16, tag="mo2")
        nc.vector.tensor_mul(mo2[:], mo[:], obs)
        smv = work.tile([128, NSTAT], BF16, tag="smv")
        vmo = mo[:].rearrange("p (c h t) -> p c t h", c=3, h=32, t=WIN)
        vms = mst.rearrange("p (h t) -> p t h", h=32, t=WIN)
        vU1 = smv[:, 0:NU1].rearrange("p (c g t) -> p c g t", c=3, g=NG, t=WIN)
        vUc = smv[:, NU1:NU1 + NUC].rearrange("p (g t) -> p g t", g=NG, t=WIN)
        with nc.allow_low_precision(reason="short class sums; bf16 ok"):
            for gi, (h0, h1_) in enumerate(H2G):
                nc.vector.reduce_sum(
                    vU1[:, :, gi, :], vmo[:, :, :, h0:h1_],
                    axis=mybir.AxisListType.X,
                )
                nc.vector.reduce_sum(
                    vUc[:, gi, :], vms[:, :, h0:h1_], axis=mybir.AxisListType.X
                )
            nc.vector.reduce_sum(
                smv[:, NSTAT - 1: NSTAT], mo2[:], axis=mybir.AxisListType.X
            )
        # class matmul: [14, NSTAT] = wclsT^T @ smv   (after encoder on PE)
        sps = ps.tile([14, NSTAT], F32, tag="stat")
        nc.tensor.matmul(sps[:], wc3[:, 84:98], smv[:], start=True, stop=True)
        outv = work.tile([14, NSTAT], F32, tag="outv")
        nc.scalar.activation(
            outv[:], sps[:], mybir.ActivationFunctionType.Identity
        )
        nc.sync.dma_start(a_out["outv"], outv[:])

        # ---------- conv1: direct 5x5 grid via W1eff variants ----------
        c1 = ps.tile([128, WIN * 5 * 5], F32, tag="c1")
        vc1 = c1[:].rearrange("p (t a b) -> p t a b", t=WIN, a=5, b=5)
        for a5 in range(5):
            for b5 in range(5):
                v = M35[a5] * 3 + M35[b5]
                for kt in range(3):
                    for u in range(2):
                        nc.tensor.matmul(
                            vc1[:, :, a5, b5],
                            wc1[:, ((v * 3 + kt) * 2 + u) * 128:
                                ((v * 3 + kt) * 2 + u + 1) * 128],
                            fpad[:, u * WP + kt: u * WP + kt + WIN],
                            start=(kt == 0 and u == 0),
                            stop=(kt == 2 and u == 1),
                        )
        # relu + b1 -> h1p interior (bf16)
        vh1p = h1p[:].rearrange("p (t a b) -> p t a b", t=WP, a=7, b=7)
        nc.scalar.activation(
            vh1p[:, 1:1 + WIN, 1:6, 1:6],
            vc1[:],
            mybir.ActivationFunctionType.Relu,
            bias=cons[:, 2:3],
        )

        # ---------- conv2: 3x3x3 on the padded 5-grid ----------
        c2 = ps.tile([64, WIN * 5 * 5], F32, tag="c2")
        for kt in range(3):
            for kh in range(3):
                for kw in range(3):
                    tap = (kt * 3 + kh) * 3 + kw
                    nc.tensor.matmul(
                        c2[:],
                        wc2[:, tap * 64: (tap + 1) * 64],
                        vh1p[:, kt:kt + WIN, kh:kh + 5, kw:kw + 5],
                        start=(tap == 0),
                        stop=(tap == 26),
                    )
        h2 = work.tile([64, WIN * 5 * 5], BF16, tag="h2")
        nc.scalar.activation(
            h2[:], c2[:], mybir.ActivationFunctionType.Relu,
            bias=cons[0:64, 3:4],
        )

        # ---------- expand 5-grid -> padded 7-grid, (b, t, a) layout ----
        vh2 = h2[:].rearrange("p (t a b) -> p t a b", t=WIN, a=5, b=5)
        vh2p = h2p[0:64, 0:WP * 81].rearrange("p (b t a) -> p t a b", b=9, t=WP, a=9)
        ci = 0
        for (da, la, sa, lsa) in G57:
            for (db, lb, sb, lsb) in G57:
                src = vh2[:, :, sa:sa + lsa, sb:sb + lsb]
                if lsa == 1 or lsb == 1:
                    src = src.broadcast_to([64, WIN, la, lb])
                dst = vh2p[:, 1:1 + WIN, 1 + da:1 + da + la, 1 + db:1 + db + lb]
                if ci % 2 == 0:
                    nc.vector.tensor_copy(dst, src)
                else:
                    nc.gpsimd.tensor_copy(dst, src)
                ci += 1

        # ---------- conv3: vox-stationary (flat 99-col slices; 2 junk
        # a-rows per tau that the host ignores), b3 folded via ones row --
        NVX = 9 * WIN  # 99
        c3 = ps.tile([NVX, 21], F32, tag="c3")
        for bc in range(7):
            for kt in range(3):
                for kh in range(3):
                    for kw in range(3):
                        tap = (kt * 3 + kh) * 3 + kw
                        rows = 65 if tap == 0 else 64
                        base = (kw + bc) * WP * 9 + kt * 9 + kh
                        nc.tensor.matmul(
                            c3[:, bc * 3: (bc + 1) * 3],
                            h2p[0:rows, base: base + NVX],
                            wc3[0:rows, tap * 3: (tap + 1) * 3],
                            start=(tap == 0),
                            stop=(tap == 26),
                        )
        recon = work.tile([NVX, 21], F32, tag="recon")
        nc.scalar.activation(
            recon[:], c3[:], mybir.ActivationFunctionType.Identity
        )
        nc.sync.dma_start(a_out["recon"], recon[:])


_CACHE = {}


def _build():
    if "nc" in _CACHE:
        return _CACHE["nc"]
    nc = bacc.Bacc("TRN2", target_bir_lowering=False, debug=False)
    a_in = {}

    def din(name, shape, dt):
        a_in[name] = nc.dram_tensor(name, shape, dt, kind="ExternalInput").ap()

    din("consts", (128, 8), F32)
    din("obsmask", (128, 1760), BF16)
    din("wencT", (4, 128, 24 * D), F8)
    din("wc1", (128, 54 * 128), F8)
    din("wc2", (128, 27 * 64), F8)
    din("wc3", (128, 98), BF16)
    a_out = {}
    for name, shape in [("recon", (9 * WIN, 21)), ("outv", (14, NSTAT))]:
        a_out[name] = nc.dram_tensor(name, shape, F32, kind="ExternalOutput").ap()
    _emit(nc, a_in, a_out)
    nc.compile()
    _CACHE["nc"] = nc
    return nc


def make_in_maps(obs_strip, mask, W_enc, b_enc, w1, b1, w2, b2, w3, b3):
    import ml_dtypes

    bf16 = ml_dtypes.bfloat16
    f8 = ml_dtypes.float8_e4m3

    obs_strip = np.asarray(obs_strip, np.float32)
    mask_f = np.asarray(mask).astype(np.float32)

    # --- shared weights ---
    wencT = np.ascontiguousarray(
        np.asarray(W_enc, np.float32)
        .reshape(D, 3, 32, 2, 64)
        .transpose(3, 4, 1, 2, 0)
        .reshape(128, 96, D)
        .reshape(128, 4, 24 * D)
        .transpose(1, 0, 2)
    ).astype(f8)

    K = {0: [1, 2], 1: [0, 1, 2], 2: [0, 1]}
    w1 = np.asarray(w1, np.float32)
    W1e = np.zeros((9, 3, 128, 2, 128), np.float32)  # [v, kt, c, u, dmod]
    for va in range(3):
        for vb in range(3):
            for kt in range(3):
                eff = w1[:, :, kt][:, :, K[va]][:, :, :, K[vb]].sum((2, 3))
                W1e[va * 3 + vb, kt] = eff.reshape(128, 2, 128)
    wc1 = np.ascontiguousarray(
        W1e.transpose(4, 0, 1, 3, 2).reshape(128, 54 * 128)
    ).astype(f8)

    wc2 = np.ascontiguousarray(
        np.asarray(w2, np.float32).transpose(1, 2, 3, 4, 0).reshape(128, 27 * 64)
    ).astype(f8)

    wc3 = np.zeros((128, 98), np.float32)
    wc3[0:64, 0:81] = np.asarray(w3, np.float32).transpose(1, 2, 3, 4, 0).reshape(64, 81)
    wc3[64, 0:3] = np.asarray(b3, np.float32)
    for u in range(2):
        for j in range(7):
            w0, w1_ = WCLS_BOUNDS[j], WCLS_BOUNDS[j + 1]
            wc3[u * 64 + w0: u * 64 + w1_, 84 + u * 7 + j] = 1.0
    wc3 = wc3.astype(bf16)

    consts = np.zeros((128, 8), np.float32)
    consts[:, 0] = np.asarray(b_enc, np.float32)[0:128]
    consts[:, 1] = np.asarray(b_enc, np.float32)[128:256]
    consts[:, 2] = np.asarray(b1, np.float32)
    consts[0:64, 3] = np.asarray(b2, np.float32)

    shared = {"wencT": wencT, "wc1": wc1, "wc2": wc2, "wc3": wc3,
              "consts": consts}

    def perm_obs(o):  # [t, C, H, W] -> [128, (c, h2, t)]
        t = o.shape[0]
        return (o.reshape(t, 3, 32, 2, 64).transpose(3, 4, 1, 2, 0)
                .reshape(128, 3 * 32 * t))

    def perm_msk(m):  # [t, H, W] -> [128, (h2, t)]
        t = m.shape[0]
        return (m.reshape(t, 32, 2, 64).transpose(2, 3, 1, 0)
                .reshape(128, 32 * t))

    in_maps = []
    for core in range(NCORES):
        b, th = core // 2, core % 2
        s = 5 * th
        om = np.zeros((128, 1760), np.float32)
        om[:, 0:1056] = perm_obs(obs_strip[b, s:s + WIN])
        om[:, 1056:1408] = perm_msk(1.0 - mask_f[b, s:s + WIN])
        mstat = mask_f[b].copy()
        if th == 0:
            mstat[8:] = 0.0
        else:
            mstat[:8] = 0.0
        om[:, 1408:1760] = perm_msk(mstat[s:s + WIN])
        in_maps.append({"obsmask": om.astype(bf16), **shared})
    return in_maps


# host-side fold: (g, hpar) -> h class contributions
HCLS_SRC = [[(0, 0)], [(0, 1)], [(1, 0)],
            [(1, 1), (2, 0), (2, 1), (3, 0)],
            [(3, 1)], [(4, 0)], [(4, 1)]]


def assemble(results):
    total_sq = 0.0
    total_cnt = 0.0
    total_s2 = 0.0
    for core in range(NCORES):
        r = results[core]
        rec = r["recon"].astype(np.float64).reshape(WIN, 9, 7, 3)[:, 0:7]  # [tau,a,b,c]
        outv = r["outv"].astype(np.float64)
        U1 = outv[:, 0:NU1].reshape(2, 7, 3, NG, WIN)   # [u,j,c,g,tau]
        Uc = outv[:, NU1:NU1 + NUC].reshape(2, 7, NG, WIN)  # [u,j,g,tau]
        total_s2 += float(outv[:, NSTAT - 1].sum())
        s1 = np.zeros((3, 7, 7, WIN))   # [c, hcls, wcls, tau]
        cnt = np.zeros((7, 7, WIN))     # [hcls, wcls, tau]
        for i in range(7):
            for (g, u) in HCLS_SRC[i]:
                s1[:, i] += U1[u, :, :, g, :].transpose(1, 0, 2)
                cnt[i] += Uc[u, :, g, :]
        rt = rec.transpose(3, 1, 2, 0)  # [c, a(hcls), b(wcls), tau]
        total_sq += float((rt * rt * cnt[None]).sum() - 2.0 * (rt * s1).sum())
        total_cnt += float(cnt.sum())
    loss = (total_sq + total_s2) / max(total_cnt * C, 1.0)
    return np.float32(loss)


def kernel(**inputs):
    nc = _build()
    in_maps = make_in_maps(**inputs)
    res = bass_utils.run_bass_kernel_spmd(nc, in_maps, core_ids=list(range(NCORES)))
    _CACHE["last_res"] = res
    return assemble(res.results)


if __name__ == "__main__":
    pass


# revision 37
# speedup vs baseline: 3.0982x; 1.0881x over previous
"""Masked video loss kernel for TRN2 (8 NeuronCores, SPMD).

Algorithmic structure exploited:
- The decoder input feat_3d is spatially constant (broadcast of per-frame
  features over H=W=64), so conv1 collapses to a per-frame linear map with
  9 edge-variant weight sums (W1eff), evaluated directly on a 5x5 class
  grid. conv2 runs as a true 3x3x3 conv on the (padded) 5-grid; its output
  is expanded to the 7-grid on which conv3 produces the 7x7 recon classes.
  All exact (class algebra), not approximations.
- Masked MSE folds through per-class stats: sum (r-o)^2 = r^2 cnt - 2 r s1
  + s2 per (t, 7x7 class); s1/cnt come from one 0/1-matrix PE matmul plus
  segmented DVE reduces.

Sharding: core = 2*b + th. Each core handles batch b and an 11-frame
t-window starting at s = 5*th (host shifts the data, so the program is
SPMD-uniform); decoder outputs are valid for the core's 8-frame t-half,
and mask stats are host-zeroed outside that half.

Precision: W_enc / w1eff / w2 in fp8-e4m3 (stationary operands), obs /
activations bf16, accumulation fp32. Measured end-to-end loss rel err
~5e-3 (gate 2e-2).
"""

import sys

sys.path.insert(0, "/opt/trn_rl_repo")

from contextlib import ExitStack  # noqa: E402

import numpy as np  # noqa: E402

import concourse.bacc as bacc  # noqa: E402
import concourse.mybir as mybir  # noqa: E402
import concourse.tile as tile  # noqa: E402
from concourse import bass_utils  # noqa: E402

B, T, C, H, W = 4, 16, 3, 64, 64
D = 256
NCORES = 8

F32 = mybir.dt.float32
BF16 = mybir.dt.bfloat16
F8 = mybir.dt.float8e4

WIN = 11          # feats/conv t-window frames per core
WP = WIN + 2      # padded window
M35 = [0, 1, 1, 1, 2]          # 5-grid pos -> 3-class variant
M57 = [0, 1, 2, 2, 2, 3, 4]    # 7-grid pos -> 5-grid src index
# expansion groups (dst0, dstlen, src0, srclen) along one axis for 5->7
G57 = [(0, 2, 0, 2), (2, 3, 2, 1), (5, 2, 3, 2)]
# h2-row groups for segmented stats reduction (h = 2*h2 + hpar)
H2G = [(0, 1), (1, 2), (2, 30), (30, 31), (31, 32)]
NG = len(H2G)
WCLS_BOUNDS = [0, 1, 2, 3, 61, 62, 63, 64]

NU1 = 3 * NG * WIN       # 165
NUC = NG * WIN           # 55
NSTAT = NU1 + NUC + 1    # 221


def _emit(nc, a_in, a_out):
    ctx = ExitStack()
    tc = tile.TileContext(nc)
    with tc, ctx:
        io = ctx.enter_context(tc.tile_pool(name="io", bufs=1))
        wkp = ctx.enter_context(tc.tile_pool(name="wkp", bufs=3))
        work = ctx.enter_context(tc.tile_pool(name="work", bufs=1))
        ps = ctx.enter_context(tc.tile_pool(name="ps", bufs=1, space="PSUM"))

        # ---------- early memsets (Pool; no deps) ----------
        fpad = work.tile([128, 2 * WP], BF16, tag="fpad")
        nc.gpsimd.memset(fpad[:], 0.0)
        h1p = work.tile([128, WP * 7 * 7], BF16, tag="h1p")
        nc.gpsimd.memset(h1p[:], 0.0)
        h2p = work.tile([65, WP * 9 * 9 + 2], BF16, tag="h2p")
        nc.gpsimd.memset(h2p[0:64, :], 0.0)
        nc.gpsimd.memset(h2p[64:65, :], 1.0)

        # ---------- input DMAs (serialized by the DMA engine) ----------
        cons = io.tile([128, 8], F32)
        nc.sync.dma_start(cons[:], a_in["consts"])
        om = io.tile([128, 1966], BF16)
        nc.sync.dma_start(om[:], a_in["obsmask"])
        obs = om[:, 0:1056]
        keep = om[:, 1056:1408]
        mst = om[:, 1408:1760]

        # ---------- PE warm-up (p-state ramp) ----------
        warm_ps = ps.tile([2, 512], F32, tag="warm")
        for i in range(24):
            nc.tensor.matmul(
                warm_ps[:, 0:8], cons[:, 0:2], cons[:, 0:8],
                start=(i == 0), stop=(i == 23),
            )
        # early act-table preload (off the critical path)
        junk = work.tile([2, 8], F32, tag="junk")
        nc.scalar.activation(
            junk[:], warm_ps[:, 0:8], mybir.ActivationFunctionType.Relu
        )
        for i in range(10):
            nc.tensor.matmul(
                warm_ps[:], om[:, 0:2], om[:, 0:512],
                start=(i == 0), stop=(i == 9),
            )

        # ---------- encoder input masking ----------
        xt = work.tile([128, 3 * 32 * WIN], BF16, tag="xt")
        for c in range(C):
            sl = slice(c * 32 * WIN, (c + 1) * 32 * WIN)
            nc.vector.tensor_mul(xt[:, sl], obs[:, sl], keep)

        # bias vectors on om row 0 (cols 1774:1902 b1, 1902:1966 b2);
        # ones row at partition 0 for psum bias preloads
        onesrow = work.tile([1, WIN * 25], BF16, tag="onesrow")
        nc.gpsimd.memset(onesrow[:], 1.0)

        # ---------- encoder matmuls: feats [d, tau] in two d-halves ----
        feats0 = ps.tile([128, WIN], F32, tag="feats0")
        feats1 = ps.tile([128, WIN], F32, tag="feats1")
        fps = [feats0, feats1]
        for g in range(4):
            wk = wkp.tile([128, 24 * D], F8, tag="wk")
            nc.sync.dma_start(wk[:], a_in["wencT"][g])
            for r in range(24):
                ki = g * 24 + r
                for u in range(2):
                    nc.tensor.matmul(
                        fps[u][:],
                        wk[:, r * D + u * 128: r * D + (u + 1) * 128],
                        xt[:, ki * WIN: (ki + 1) * WIN],
                        start=(ki == 0),
                        stop=(ki == 95),
                    )

        # conv weights after wencT (conv chain is later anyway); wc1 in two
        # pieces so conv1 phase A starts on the first
        wc1 = io.tile([128, 54 * 128], F8)
        nc.sync.dma_start(wc1[:, 0:36 * 128], a_in["wc1"][:, 0:36 * 128])
        nc.sync.dma_start(wc1[:, 36 * 128:], a_in["wc1"][:, 36 * 128:])
        wc2 = io.tile([128, 27 * 64], F8)
        nc.sync.dma_start(wc2[:], a_in["wc2"])
        wc3 = io.tile([128, 98], BF16)
        nc.sync.dma_start(wc3[:], a_in["wc3"])

        # feats + b_enc (per-partition broadcast add) -> fpad on DVE
        for u in range(2):
            nc.vector.tensor_add(
                fpad[:, u * WP + 1: u * WP + 1 + WIN], fps[u][:],
                cons[:, u: u + 1].broadcast_to([128, WIN]),
            )

        # ---------- mask stats (DVE; overlapped with DMA/encoder) ------
        mo = work.tile([128, 1056], BF16, tag="mo")
        for c in range(C):
            sl = slice(c * 32 * WIN, (c + 1) * 32 * WIN)
            nc.vector.tensor_mul(mo[:, sl], obs[:, sl], mst)
        mo2 = work.tile([128, 1056], BF16, tag="mo2")
        nc.vector.tensor_mul(mo2[:], mo[:], obs)
        smv = work.tile([128, NSTAT], BF16, tag="smv")
        vmo = mo[:].rearrange("p (c h t) -> p c t h", c=3, h=32, t=WIN)
        vms = mst.rearrange("p (h t) -> p t h", h=32, t=WIN)
        vU1 = smv[:, 0:NU1].rearrange("p (c g t) -> p c g t", c=3, g=NG, t=WIN)
        vUc = smv[:, NU1:NU1 + NUC].rearrange("p (g t) -> p g t", g=NG, t=WIN)
        with nc.allow_low_precision(reason="short class sums; bf16 ok"):
            for gi, (h0, h1_) in enumerate(H2G):
                nc.vector.reduce_sum(
                    vU1[:, :, gi, :], vmo[:, :, :, h0:h1_],
                    axis=mybir.AxisListType.X,
                )
                nc.vector.reduce_sum(
                    vUc[:, gi, :], vms[:, :, h0:h1_], axis=mybir.AxisListType.X
                )
            nc.vector.reduce_sum(
                smv[:, NSTAT - 1: NSTAT], mo2[:], axis=mybir.AxisListType.X
            )
        # class matmul: [14, NSTAT] = wclsT^T @ smv   (after encoder on PE)
        sps = ps.tile([14, NSTAT], F32, tag="stat")
        nc.tensor.matmul(sps[:], om[:, 1760:1774], smv[:], start=True, stop=True)
        outv = work.tile([14, NSTAT], F32, tag="outv")
        nc.scalar.activation(
            outv[:], sps[:], mybir.ActivationFunctionType.Identity
        )
        nc.sync.dma_start(a_out["outv"], outv[:])

        # ---------- conv1: direct 5x5 grid via W1eff variants ----------
        # phase A groups use variants in the first wc1 piece (v <= 4)
        groups = sorted(
            ((a5, b5) for a5 in range(5) for b5 in range(5)),
            key=lambda ab: (M35[ab[0]] * 3 + M35[ab[1]] > 5,),
        )
        c1 = ps.tile([128, WIN * 5 * 5], F32, tag="c1")
        vc1 = c1[:].rearrange("p (t a b) -> p t a b", t=WIN, a=5, b=5)
        for a5, b5 in groups:
            v = M35[a5] * 3 + M35[b5]
            nc.tensor.matmul(
                vc1[:, :, a5, b5], om[0:1, 1774:1902], onesrow[:, 0:WIN],
                start=True, stop=False,
            )
            for kt in range(3):
                for u in range(2):
                    nc.tensor.matmul(
                        vc1[:, :, a5, b5],
                        wc1[:, ((v * 3 + kt) * 2 + u) * 128:
                            ((v * 3 + kt) * 2 + u + 1) * 128],
                        fpad[:, u * WP + kt: u * WP + kt + WIN],
                        start=False,
                        stop=(kt == 2 and u == 1),
                    )
        # relu -> h1p interior (bf16), split across Act and DVE
        vh1p = h1p[:].rearrange("p (t a b) -> p t a b", t=WP, a=7, b=7)
        nc.vector.tensor_relu(
            vh1p[:, 1:1 + WIN, 1:6, 1:6], vc1[:]
        )

        # ---------- conv2: 3x3x3 on the padded 5-grid (kh-outer) ----------
        c2 = ps.tile([64, WIN * 5 * 5], F32, tag="c2")
        nc.tensor.matmul(
            c2[:], om[0:1, 1902:1966], onesrow[:, 0:WIN * 25],
            start=True, stop=False,
        )
        for kh in range(3):
            for kt in range(3):
                for kw in range(3):
                    tap = (kt * 3 + kh) * 3 + kw
                    nc.tensor.matmul(
                        c2[:],
                        wc2[:, tap * 64: (tap + 1) * 64],
                        vh1p[:, kt:kt + WIN, kh:kh + 5, kw:kw + 5],
                        start=False,
                        stop=(kh == 2 and kt == 2 and kw == 2),
                    )
        # relu -> h2 (bf16) on DVE, then 5->7 expansion copies across
        # engines; db-group order so conv3's bc=0 unblocks first
        h2 = work.tile([64, WIN * 5 * 5], BF16, tag="h2")
        nc.vector.tensor_relu(h2[:], c2[:])
        vh2 = h2[:].rearrange("p (t a b) -> p t a b", t=WIN, a=5, b=5)
        vh2p = h2p[0:64, 0:WP * 81].rearrange("p (b t a) -> p t a b", b=9, t=WP, a=9)
        cpeng = ["pool", "vector", "pool", "pool", "vector", "vector",
                 "scalar", "vector", "scalar"]
        ci = 0
        for (db, lb, sb, lsb) in G57:
            for (da, la, sa, lsa) in G57:
                src = vh2[:, :, sa:sa + lsa, sb:sb + lsb]
                if lsa == 1 or lsb == 1:
                    src = src.broadcast_to([64, WIN, la, lb])
                dst = vh2p[:, 1:1 + WIN, 1 + da:1 + da + la, 1 + db:1 + db + lb]
                eng = cpeng[ci]
                if eng == "vector":
                    nc.vector.tensor_copy(dst, src)
                elif eng == "pool":
                    nc.gpsimd.tensor_copy(dst, src)
                else:
                    nc.scalar.copy(dst, src)
                ci += 1

        # ---------- conv3: vox-stationary (flat 99-col slices; 2 junk
        # a-rows per tau that the host ignores), b3 folded via ones row --
        NVX = 9 * WIN  # 99
        c3 = ps.tile([NVX, 21], F32, tag="c3")
        for bc in range(7):
            for kt in range(3):
                for kh in range(3):
                    for kw in range(3):
                        tap = (kt * 3 + kh) * 3 + kw
                        rows = 65 if tap == 0 else 64
                        base = (kw + bc) * WP * 9 + kt * 9 + kh
                        nc.tensor.matmul(
                            c3[:, bc * 3: (bc + 1) * 3],
                            h2p[0:rows, base: base + NVX],
                            wc3[0:rows, tap * 3: (tap + 1) * 3],
                            start=(tap == 0),
                            stop=(tap == 26),
                        )
        recon = work.tile([NVX, 21], F32, tag="recon")
        nc.vector.tensor_copy(recon[:], c3[:])
        nc.sync.dma_start(a_out["recon"], recon[:])


_CACHE = {}


def _build():
    if "nc" in _CACHE:
        return _CACHE["nc"]
    nc = bacc.Bacc("TRN2", target_bir_lowering=False, debug=False)
    a_in = {}

    def din(name, shape, dt):
        a_in[name] = nc.dram_tensor(name, shape, dt, kind="ExternalInput").ap()

    din("consts", (128, 8), F32)
    din("obsmask", (128, 1966), BF16)
    din("wencT", (4, 128, 24 * D), F8)
    din("wc1", (128, 54 * 128), F8)
    din("wc2", (128, 27 * 64), F8)
    din("wc3", (128, 98), BF16)
    a_out = {}
    for name, shape in [("recon", (9 * WIN, 21)), ("outv", (14, NSTAT))]:
        a_out[name] = nc.dram_tensor(name, shape, F32, kind="ExternalOutput").ap()
    _emit(nc, a_in, a_out)
    nc.compile()
    _CACHE["nc"] = nc
    return nc


def make_in_maps(obs_strip, mask, W_enc, b_enc, w1, b1, w2, b2, w3, b3):
    import ml_dtypes

    bf16 = ml_dtypes.bfloat16
    f8 = ml_dtypes.float8_e4m3

    obs_strip = np.asarray(obs_strip, np.float32)
    mask_f = np.asarray(mask).astype(np.float32)

    # --- shared weights ---
    wencT = np.ascontiguousarray(
        np.asarray(W_enc, np.float32)
        .reshape(D, 3, 32, 2, 64)
        .transpose(3, 4, 1, 2, 0)
        .reshape(128, 96, D)
        .reshape(128, 4, 24 * D)
        .transpose(1, 0, 2)
    ).astype(f8)

    K = {0: [1, 2], 1: [0, 1, 2], 2: [0, 1]}
    w1 = np.asarray(w1, np.float32)
    W1e = np.zeros((9, 3, 128, 2, 128), np.float32)  # [v, kt, c, u, dmod]
    for va in range(3):
        for vb in range(3):
            for kt in range(3):
                eff = w1[:, :, kt][:, :, K[va]][:, :, :, K[vb]].sum((2, 3))
                W1e[va * 3 + vb, kt] = eff.reshape(128, 2, 128)
    wc1 = np.ascontiguousarray(
        W1e.transpose(4, 0, 1, 3, 2).reshape(128, 54 * 128)
    ).astype(f8)

    wc2 = np.ascontiguousarray(
        np.asarray(w2, np.float32).transpose(1, 2, 3, 4, 0).reshape(128, 27 * 64)
    ).astype(f8)

    wc3 = np.zeros((128, 98), np.float32)
    wc3[0:64, 0:81] = np.asarray(w3, np.float32).transpose(1, 2, 3, 4, 0).reshape(64, 81)
    wc3[64, 0:3] = np.asarray(b3, np.float32)
    wc3 = wc3.astype(bf16)
    wcls = np.zeros((128, 14), np.float32)
    for u in range(2):
        for j in range(7):
            w0, w1_ = WCLS_BOUNDS[j], WCLS_BOUNDS[j + 1]
            wcls[u * 64 + w0: u * 64 + w1_, u * 7 + j] = 1.0

    consts = np.zeros((128, 8), np.float32)
    consts[:, 0] = np.asarray(b_enc, np.float32)[0:128]
    consts[:, 1] = np.asarray(b_enc, np.float32)[128:256]
    consts[:, 2] = np.asarray(b1, np.float32)
    consts[0:64, 3] = np.asarray(b2, np.float32)

    shared = {"wencT": wencT, "wc1": wc1, "wc2": wc2, "wc3": wc3,
              "consts": consts}

    def perm_obs(o):  # [t, C, H, W] -> [128, (c, h2, t)]
        t = o.shape[0]
        return (o.reshape(t, 3, 32, 2, 64).transpose(3, 4, 1, 2, 0)
                .reshape(128, 3 * 32 * t))

    def perm_msk(m):  # [t, H, W] -> [128, (h2, t)]
        t = m.shape[0]
        return (m.reshape(t, 32, 2, 64).transpose(2, 3, 1, 0)
                .reshape(128, 32 * t))

    in_maps = []
    for core in range(NCORES):
        b, th = core // 2, core % 2
        s = 5 * th
        om = np.zeros((128, 1966), np.float32)
        om[:, 1760:1774] = wcls
        om[0, 1774:1902] = np.asarray(b1, np.float32)
        om[0, 1902:1966] = np.asarray(b2, np.float32)
        om[:, 0:1056] = perm_obs(obs_strip[b, s:s + WIN])
        om[:, 1056:1408] = perm_msk(1.0 - mask_f[b, s:s + WIN])
        mstat = mask_f[b].copy()
        if th == 0:
            mstat[8:] = 0.0
        else:
            mstat[:8] = 0.0
        om[:, 1408:1760] = perm_msk(mstat[s:s + WIN])
        in_maps.append({"obsmask": om.astype(bf16), **shared})
    return in_maps


# host-side fold: (g, hpar) -> h class contributions
HCLS_SRC = [[(0, 0)], [(0, 1)], [(1, 0)],
            [(1, 1), (2, 0), (2, 1), (3, 0)],
            [(3, 1)], [(4, 0)], [(4, 1)]]


def assemble(results):
    total_sq = 0.0
    total_cnt = 0.0
    total_s2 = 0.0
    for core in range(NCORES):
        r = results[core]
        rec = r["recon"].astype(np.float64).reshape(WIN, 9, 7, 3)[:, 0:7]  # [tau,a,b,c]
        outv = r["outv"].astype(np.float64)
        U1 = outv[:, 0:NU1].reshape(2, 7, 3, NG, WIN)   # [u,j,c,g,tau]
        Uc = outv[:, NU1:NU1 + NUC].reshape(2, 7, NG, WIN)  # [u,j,g,tau]
        total_s2 += float(outv[:, NSTAT - 1].sum())
        s1 = np.zeros((3, 7, 7, WIN))   # [c, hcls, wcls, tau]
        cnt = np.zeros((7, 7, WIN))     # [hcls, wcls, tau]
        for i in range(7):
            for (g, u) in HCLS_SRC[i]:
                s1[:, i] += U1[u, :, :, g, :].transpose(1, 0, 2)
                cnt[i] += Uc[u, :, g, :]
        rt = rec.transpose(3, 1, 2, 0)  # [c, a(hcls), b(wcls), tau]
        total_sq += float((rt * rt * cnt[None]).sum() - 2.0 * (rt * s1).sum())
        total_cnt += float(cnt.sum())
    loss = (total_sq + total_s2) / max(total_cnt * C, 1.0)
    return np.float32(loss)


def kernel(**inputs):
    nc = _build()
    in_maps = make_in_maps(**inputs)
    res = bass_utils.run_bass_kernel_spmd(nc, in_maps, core_ids=list(range(NCORES)))
    _CACHE["last_res"] = res
    return assemble(res.results)


if __name__ == "__main__":
    pass


# revision 39
# speedup vs baseline: 3.1375x; 1.0127x over previous
"""Masked video loss kernel for TRN2 (8 NeuronCores, SPMD).

Algorithmic structure exploited:
- The decoder input feat_3d is spatially constant (broadcast of per-frame
  features over H=W=64), so conv1 collapses to a per-frame linear map with
  9 edge-variant weight sums (W1eff), evaluated directly on a 5x5 class
  grid. conv2 runs as a true 3x3x3 conv on the (padded) 5-grid; its output
  is expanded to the 7-grid on which conv3 produces the 7x7 recon classes.
  All exact (class algebra), not approximations.
- Masked MSE folds through per-class stats: sum (r-o)^2 = r^2 cnt - 2 r s1
  + s2 per (t, 7x7 class); s1/cnt come from one 0/1-matrix PE matmul plus
  segmented DVE reduces.

Sharding: core = 2*b + th. Each core handles batch b and an 11-frame
t-window starting at s = 5*th (host shifts the data, so the program is
SPMD-uniform); decoder outputs are valid for the core's 8-frame t-half,
and mask stats are host-zeroed outside that half.

Precision: W_enc / w1eff / w2 in fp8-e4m3 (stationary operands), obs /
activations bf16, accumulation fp32. Measured end-to-end loss rel err
~5e-3 (gate 2e-2).
"""

import sys

sys.path.insert(0, "/opt/trn_rl_repo")

from contextlib import ExitStack  # noqa: E402

import numpy as np  # noqa: E402

import concourse.bacc as bacc  # noqa: E402
import concourse.mybir as mybir  # noqa: E402
import concourse.tile as tile  # noqa: E402
from concourse import bass_utils  # noqa: E402

B, T, C, H, W = 4, 16, 3, 64, 64
D = 256
NCORES = 8

F32 = mybir.dt.float32
BF16 = mybir.dt.bfloat16
F8 = mybir.dt.float8e4

WIN = 11          # feats/conv t-window frames per core
WP = WIN + 2      # padded window
M35 = [0, 1, 1, 1, 2]          # 5-grid pos -> 3-class variant
M57 = [0, 1, 2, 2, 2, 3, 4]    # 7-grid pos -> 5-grid src index
# expansion groups (dst0, dstlen, src0, srclen) along one axis for 5->7
G57 = [(0, 2, 0, 2), (2, 3, 2, 1), (5, 2, 3, 2)]
# h2-row groups for segmented stats reduction (h = 2*h2 + hpar)
H2G = [(0, 1), (1, 2), (2, 30), (30, 31), (31, 32)]
NG = len(H2G)
WCLS_BOUNDS = [0, 1, 2, 3, 61, 62, 63, 64]

NU1 = 3 * NG * WIN       # 165
NUC = NG * WIN           # 55
NSTAT = NU1 + NUC + 1    # 221


def _emit(nc, a_in, a_out):
    ctx = ExitStack()
    tc = tile.TileContext(nc)
    with tc, ctx:
        io = ctx.enter_context(tc.tile_pool(name="io", bufs=1))
        wkp = ctx.enter_context(tc.tile_pool(name="wkp", bufs=3))
        work = ctx.enter_context(tc.tile_pool(name="work", bufs=1))
        ps = ctx.enter_context(tc.tile_pool(name="ps", bufs=1, space="PSUM"))

        # ---------- early memsets (Pool; no deps) ----------
        fpad = work.tile([128, 2 * WP], BF16, tag="fpad")
        nc.gpsimd.memset(fpad[:], 0.0)
        h1p = work.tile([128, WP * 7 * 7], BF16, tag="h1p")
        nc.gpsimd.memset(h1p[:], 0.0)
        h2p = work.tile([65, WP * 9 * 9 + 2], BF16, tag="h2p")
        nc.gpsimd.memset(h2p[0:64, :], 0.0)
        nc.gpsimd.memset(h2p[64:65, :], 1.0)

        # ---------- input DMAs (serialized by the DMA engine) ----------
        cons = io.tile([128, 8], F32)
        nc.sync.dma_start(cons[:], a_in["consts"])
        om = io.tile([128, 1262], BF16)
        nc.sync.dma_start(om[:], a_in["obsmask"])
        obs = om[:, 0:1056]
        msk8 = io.tile([128, 704], mybir.dt.uint8)
        nc.sync.dma_start(msk8[:], a_in["msk"])
        kmbf = work.tile([128, 704], BF16, tag="kmbf")
        nc.vector.tensor_copy(kmbf[:], msk8[:])
        keep = kmbf[:, 0:352]
        mst = kmbf[:, 352:704]

        # ---------- PE warm-up (p-state ramp) ----------
        warm_ps = ps.tile([2, 512], F32, tag="warm")
        for i in range(24):
            nc.tensor.matmul(
                warm_ps[:, 0:8], cons[:, 0:2], cons[:, 0:8],
                start=(i == 0), stop=(i == 23),
            )
        # early act-table preload (off the critical path)
        junk = work.tile([2, 8], F32, tag="junk")
        nc.scalar.activation(
            junk[:], warm_ps[:, 0:8], mybir.ActivationFunctionType.Relu
        )
        for i in range(10):
            nc.tensor.matmul(
                warm_ps[:], om[:, 0:2], om[:, 0:512],
                start=(i == 0), stop=(i == 9),
            )

        # ---------- encoder input masking ----------
        xt = work.tile([128, 3 * 32 * WIN], BF16, tag="xt")
        for c in range(C):
            sl = slice(c * 32 * WIN, (c + 1) * 32 * WIN)
            nc.vector.tensor_mul(xt[:, sl], obs[:, sl], keep)

        # bias vectors on om row 0 (cols 1774:1902 b1, 1902:1966 b2);
        # ones row at partition 0 for psum bias preloads
        onesrow = work.tile([1, WIN * 25], BF16, tag="onesrow")
        nc.gpsimd.memset(onesrow[:], 1.0)

        # ---------- encoder matmuls: feats [d, tau] in two d-halves ----
        feats0 = ps.tile([128, WIN], F32, tag="feats0")
        feats1 = ps.tile([128, WIN], F32, tag="feats1")
        fps = [feats0, feats1]
        for g in range(4):
            wk = wkp.tile([128, 24 * D], F8, tag="wk")
            nc.sync.dma_start(wk[:], a_in["wencT"][g])
            for r in range(24):
                ki = g * 24 + r
                for u in range(2):
                    nc.tensor.matmul(
                        fps[u][:],
                        wk[:, r * D + u * 128: r * D + (u + 1) * 128],
                        xt[:, ki * WIN: (ki + 1) * WIN],
                        start=(ki == 0),
                        stop=(ki == 95),
                    )

        # conv weights after wencT (conv chain is later anyway); wc1 in two
        # pieces so conv1 phase A starts on the first
        wc1 = io.tile([128, 54 * 128], F8)
        wc2 = io.tile([128, 27 * 64], F8)
        wc3 = io.tile([128, 98], BF16)
        nc.sync.dma_start(wc1[:, 0:36 * 128], a_in["wc1"][:, 0:36 * 128])
        nc.sync.dma_start(wc2[:], a_in["wc2"])
        nc.sync.dma_start(wc1[:, 36 * 128:], a_in["wc1"][:, 36 * 128:])
        nc.sync.dma_start(wc3[:], a_in["wc3"])

        # feats + b_enc (per-partition broadcast add) -> fpad on DVE
        for u in range(2):
            nc.vector.tensor_add(
                fpad[:, u * WP + 1: u * WP + 1 + WIN], fps[u][:],
                cons[:, u: u + 1].broadcast_to([128, WIN]),
            )

        # ---------- mask stats (DVE; overlapped with DMA/encoder) ------
        mo = work.tile([128, 1056], BF16, tag="mo")
        for c in range(C):
            sl = slice(c * 32 * WIN, (c + 1) * 32 * WIN)
            nc.vector.tensor_mul(mo[:, sl], obs[:, sl], mst)
        mo2 = work.tile([128, 1056], BF16, tag="mo2")
        nc.vector.tensor_mul(mo2[:], mo[:], obs)
        smv = work.tile([128, NSTAT], BF16, tag="smv")
        vmo = mo[:].rearrange("p (c h t) -> p c t h", c=3, h=32, t=WIN)
        vms = mst.rearrange("p (h t) -> p t h", h=32, t=WIN)
        vU1 = smv[:, 0:NU1].rearrange("p (c g t) -> p c g t", c=3, g=NG, t=WIN)
        vUc = smv[:, NU1:NU1 + NUC].rearrange("p (g t) -> p g t", g=NG, t=WIN)
        with nc.allow_low_precision(reason="short class sums; bf16 ok"):
            for gi, (h0, h1_) in enumerate(H2G):
                nc.vector.reduce_sum(
                    vU1[:, :, gi, :], vmo[:, :, :, h0:h1_],
                    axis=mybir.AxisListType.X,
                )
                nc.vector.reduce_sum(
                    vUc[:, gi, :], vms[:, :, h0:h1_], axis=mybir.AxisListType.X
                )
            nc.vector.reduce_sum(
                smv[:, NSTAT - 1: NSTAT], mo2[:], axis=mybir.AxisListType.X
            )
        # class matmul: [14, NSTAT] = wclsT^T @ smv   (after encoder on PE)
        sps = ps.tile([14, NSTAT], F32, tag="stat")
        nc.tensor.matmul(sps[:], om[:, 1056:1070], smv[:], start=True, stop=True)
        outv = work.tile([14, NSTAT], F32, tag="outv")
        nc.scalar.activation(
            outv[:], sps[:], mybir.ActivationFunctionType.Identity
        )
        nc.sync.dma_start(a_out["outv"], outv[:])

        # ---------- conv1: direct 5x5 grid via W1eff variants ----------
        # phase A groups use variants in the first wc1 piece (v <= 4)
        groups = sorted(
            ((a5, b5) for a5 in range(5) for b5 in range(5)),
            key=lambda ab: (M35[ab[0]] * 3 + M35[ab[1]] > 5,),
        )
        c1 = ps.tile([128, WIN * 5 * 5], F32, tag="c1")
        vc1 = c1[:].rearrange("p (t a b) -> p t a b", t=WIN, a=5, b=5)
        vh1p = h1p[:].rearrange("p (t a b) -> p t a b", t=WP, a=7, b=7)
        for gi, (a5, b5) in enumerate(groups):
            v = M35[a5] * 3 + M35[b5]
            nc.tensor.matmul(
                vc1[:, :, a5, b5], om[0:1, 1070:1198], onesrow[:, 0:WIN],
                start=True, stop=False,
            )
            for kt in range(3):
                for u in range(2):
                    nc.tensor.matmul(
                        vc1[:, :, a5, b5],
                        wc1[:, ((v * 3 + kt) * 2 + u) * 128:
                            ((v * 3 + kt) * 2 + u + 1) * 128],
                        fpad[:, u * WP + kt: u * WP + kt + WIN],
                        start=False,
                        stop=(kt == 2 and u == 1),
                    )
            if gi == 19:
                # phase A (a5 0..3) complete: relu that region now so
                # conv2's kh=0 taps need not wait for phase B
                nc.vector.tensor_relu(
                    vh1p[:, 1:1 + WIN, 1:5, 1:6], vc1[:, :, 0:4, :]
                )
        # phase B region (a5 = 4)
        nc.vector.tensor_relu(
            vh1p[:, 1:1 + WIN, 5:6, 1:6], vc1[:, :, 4:5, :]
        )

        # ---------- conv2: 3x3x3 on the padded 5-grid (kh-outer) ----------
        c2 = ps.tile([64, WIN * 5 * 5], F32, tag="c2")
        nc.tensor.matmul(
            c2[:], om[0:1, 1198:1262], onesrow[:, 0:WIN * 25],
            start=True, stop=False,
        )
        for kh in range(3):
            for kt in range(3):
                for kw in range(3):
                    tap = (kt * 3 + kh) * 3 + kw
                    nc.tensor.matmul(
                        c2[:],
                        wc2[:, tap * 64: (tap + 1) * 64],
                        vh1p[:, kt:kt + WIN, kh:kh + 5, kw:kw + 5],
                        start=False,
                        stop=(kh == 2 and kt == 2 and kw == 2),
                    )
        # relu -> h2 (bf16) on DVE, then 5->7 expansion copies across
        # engines; db-group order so conv3's bc=0 unblocks first
        h2 = work.tile([64, WIN * 5 * 5], BF16, tag="h2")
        nc.vector.tensor_relu(h2[:], c2[:])
        vh2 = h2[:].rearrange("p (t a b) -> p t a b", t=WIN, a=5, b=5)
        vh2p = h2p[0:64, 0:WP * 81].rearrange("p (b t a) -> p t a b", b=9, t=WP, a=9)
        cpeng = ["pool", "vector", "pool", "pool", "vector", "vector",
                 "scalar", "vector", "scalar"]
        ci = 0
        for (db, lb, sb, lsb) in G57:
            for (da, la, sa, lsa) in G57:
                src = vh2[:, :, sa:sa + lsa, sb:sb + lsb]
                if lsa == 1 or lsb == 1:
                    src = src.broadcast_to([64, WIN, la, lb])
                dst = vh2p[:, 1:1 + WIN, 1 + da:1 + da + la, 1 + db:1 + db + lb]
                eng = cpeng[ci]
                if eng == "vector":
                    nc.vector.tensor_copy(dst, src)
                elif eng == "pool":
                    nc.gpsimd.tensor_copy(dst, src)
                else:
                    nc.scalar.copy(dst, src)
                ci += 1

        # ---------- conv3: vox-stationary (flat 99-col slices; 2 junk
        # a-rows per tau that the host ignores), b3 folded via ones row --
        NVX = 9 * WIN  # 99
        c3 = ps.tile([NVX, 21], F32, tag="c3")
        for bc in range(7):
            for kt in range(3):
                for kh in range(3):
                    for kw in range(3):
                        tap = (kt * 3 + kh) * 3 + kw
                        rows = 65 if tap == 0 else 64
                        base = (kw + bc) * WP * 9 + kt * 9 + kh
                        nc.tensor.matmul(
                            c3[:, bc * 3: (bc + 1) * 3],
                            h2p[0:rows, base: base + NVX],
                            wc3[0:rows, tap * 3: (tap + 1) * 3],
                            start=(tap == 0),
                            stop=(tap == 26),
                        )
        recon = work.tile([NVX, 21], F32, tag="recon")
        nc.vector.tensor_copy(recon[:], c3[:])
        nc.sync.dma_start(a_out["recon"], recon[:])


_CACHE = {}


def _build():
    if "nc" in _CACHE:
        return _CACHE["nc"]
    nc = bacc.Bacc("TRN2", target_bir_lowering=False, debug=False)
    a_in = {}

    def din(name, shape, dt):
        a_in[name] = nc.dram_tensor(name, shape, dt, kind="ExternalInput").ap()

    din("consts", (128, 8), F32)
    din("obsmask", (128, 1262), BF16)
    din("msk", (128, 704), mybir.dt.uint8)
    din("wencT", (4, 128, 24 * D), F8)
    din("wc1", (128, 54 * 128), F8)
    din("wc2", (128, 27 * 64), F8)
    din("wc3", (128, 98), BF16)
    a_out = {}
    for name, shape in [("recon", (9 * WIN, 21)), ("outv", (14, NSTAT))]:
        a_out[name] = nc.dram_tensor(name, shape, F32, kind="ExternalOutput").ap()
    _emit(nc, a_in, a_out)
    nc.compile()
    _CACHE["nc"] = nc
    return nc


def make_in_maps(obs_strip, mask, W_enc, b_enc, w1, b1, w2, b2, w3, b3):
    import ml_dtypes

    bf16 = ml_dtypes.bfloat16
    f8 = ml_dtypes.float8_e4m3

    obs_strip = np.asarray(obs_strip, np.float32)
    mask_f = np.asarray(mask).astype(np.float32)

    # --- shared weights ---
    wencT = np.ascontiguousarray(
        np.asarray(W_enc, np.float32)
        .reshape(D, 3, 32, 2, 64)
        .transpose(3, 4, 1, 2, 0)
        .reshape(128, 96, D)
        .reshape(128, 4, 24 * D)
        .transpose(1, 0, 2)
    ).astype(f8)

    K = {0: [1, 2], 1: [0, 1, 2], 2: [0, 1]}
    w1 = np.asarray(w1, np.float32)
    W1e = np.zeros((9, 3, 128, 2, 128), np.float32)  # [v, kt, c, u, dmod]
    for va in range(3):
        for vb in range(3):
            for kt in range(3):
                eff = w1[:, :, kt][:, :, K[va]][:, :, :, K[vb]].sum((2, 3))
                W1e[va * 3 + vb, kt] = eff.reshape(128, 2, 128)
    wc1 = np.ascontiguousarray(
        W1e.transpose(4, 0, 1, 3, 2).reshape(128, 54 * 128)
    ).astype(f8)

    wc2 = np.ascontiguousarray(
        np.asarray(w2, np.float32).transpose(1, 2, 3, 4, 0).reshape(128, 27 * 64)
    ).astype(f8)

    wc3 = np.zeros((128, 98), np.float32)
    wc3[0:64, 0:81] = np.asarray(w3, np.float32).transpose(1, 2, 3, 4, 0).reshape(64, 81)
    wc3[64, 0:3] = np.asarray(b3, np.float32)
    wc3 = wc3.astype(bf16)
    wcls = np.zeros((128, 14), np.float32)
    for u in range(2):
        for j in range(7):
            w0, w1_ = WCLS_BOUNDS[j], WCLS_BOUNDS[j + 1]
            wcls[u * 64 + w0: u * 64 + w1_, u * 7 + j] = 1.0

    consts = np.zeros((128, 8), np.float32)
    consts[:, 0] = np.asarray(b_enc, np.float32)[0:128]
    consts[:, 1] = np.asarray(b_enc, np.float32)[128:256]
    consts[:, 2] = np.asarray(b1, np.float32)
    consts[0:64, 3] = np.asarray(b2, np.float32)

    shared = {"wencT": wencT, "wc1": wc1, "wc2": wc2, "wc3": wc3,
              "consts": consts}

    def perm_obs(o):  # [t, C, H, W] -> [128, (c, h2, t)]
        t = o.shape[0]
        return (o.reshape(t, 3, 32, 2, 64).transpose(3, 4, 1, 2, 0)
                .reshape(128, 3 * 32 * t))

    def perm_msk(m):  # [t, H, W] -> [128, (h2, t)]
        t = m.shape[0]
        return (m.reshape(t, 32, 2, 64).transpose(2, 3, 1, 0)
                .reshape(128, 32 * t))

    in_maps = []
    for core in range(NCORES):
        b, th = core // 2, core % 2
        s = 5 * th
        om = np.zeros((128, 1262), np.float32)
        om[:, 1056:1070] = wcls
        om[0, 1070:1198] = np.asarray(b1, np.float32)
        om[0, 1198:1262] = np.asarray(b2, np.float32)
        om[:, 0:1056] = perm_obs(obs_strip[b, s:s + WIN])
        msk = np.zeros((128, 704), np.uint8)
        msk[:, 0:352] = perm_msk(1.0 - mask_f[b, s:s + WIN])
        mstat = mask_f[b].copy()
        if th == 0:
            mstat[8:] = 0.0
        else:
            mstat[:8] = 0.0
        msk[:, 352:704] = perm_msk(mstat[s:s + WIN])
        in_maps.append({"obsmask": om.astype(bf16), "msk": msk, **shared})
    return in_maps


# host-side fold: (g, hpar) -> h class contributions
HCLS_SRC = [[(0, 0)], [(0, 1)], [(1, 0)],
            [(1, 1), (2, 0), (2, 1), (3, 0)],
            [(3, 1)], [(4, 0)], [(4, 1)]]


def assemble(results):
    total_sq = 0.0
    total_cnt = 0.0
    total_s2 = 0.0
    for core in range(NCORES):
        r = results[core]
        rec = r["recon"].astype(np.float64).reshape(WIN, 9, 7, 3)[:, 0:7]  # [tau,a,b,c]
        outv = r["outv"].astype(np.float64)
        U1 = outv[:, 0:NU1].reshape(2, 7, 3, NG, WIN)   # [u,j,c,g,tau]
        Uc = outv[:, NU1:NU1 + NUC].reshape(2, 7, NG, WIN)  # [u,j,g,tau]
        total_s2 += float(outv[:, NSTAT - 1].sum())
        s1 = np.zeros((3, 7, 7, WIN))   # [c, hcls, wcls, tau]
        cnt = np.zeros((7, 7, WIN))     # [hcls, wcls, tau]
        for i in range(7):
            for (g, u) in HCLS_SRC[i]:
                s1[:, i] += U1[u, :, :, g, :].transpose(1, 0, 2)
                cnt[i] += Uc[u, :, g, :]
        rt = rec.transpose(3, 1, 2, 0)  # [c, a(hcls), b(wcls), tau]
        total_sq += float((rt * rt * cnt[None]).sum() - 2.0 * (rt * s1).sum())
        total_cnt += float(cnt.sum())
    loss = (total_sq + total_s2) / max(total_cnt * C, 1.0)
    return np.float32(loss)


def kernel(**inputs):
    nc = _build()
    in_maps = make_in_maps(**inputs)
    res = bass_utils.run_bass_kernel_spmd(nc, in_maps, core_ids=list(range(NCORES)))
    _CACHE["last_res"] = res
    return assemble(res.results)


if __name__ == "__main__":
    pass


# revision 40
# speedup vs baseline: 3.1416x; 1.0013x over previous
"""Masked video loss kernel for TRN2 (8 NeuronCores, SPMD).

Algorithmic structure exploited:
- The decoder input feat_3d is spatially constant (broadcast of per-frame
  features over H=W=64), so conv1 collapses to a per-frame linear map with
  9 edge-variant weight sums (W1eff), evaluated directly on a 5x5 class
  grid. conv2 runs as a true 3x3x3 conv on the (padded) 5-grid; its output
  is expanded to the 7-grid on which conv3 produces the 7x7 recon classes.
  All exact (class algebra), not approximations.
- Masked MSE folds through per-class stats: sum (r-o)^2 = r^2 cnt - 2 r s1
  + s2 per (t, 7x7 class); s1/cnt come from one 0/1-matrix PE matmul plus
  segmented DVE reduces.

Sharding: core = 2*b + th. Each core handles batch b and an 11-frame
t-window starting at s = 5*th (host shifts the data, so the program is
SPMD-uniform); decoder outputs are valid for the core's 8-frame t-half,
and mask stats are host-zeroed outside that half.

Precision: W_enc / w1eff / w2 in fp8-e4m3 (stationary operands), obs /
activations bf16, accumulation fp32. Measured end-to-end loss rel err
~5e-3 (gate 2e-2).
"""

import sys

sys.path.insert(0, "/opt/trn_rl_repo")

from contextlib import ExitStack  # noqa: E402

import numpy as np  # noqa: E402

import concourse.bacc as bacc  # noqa: E402
import concourse.mybir as mybir  # noqa: E402
import concourse.tile as tile  # noqa: E402
from concourse import bass_utils  # noqa: E402

B, T, C, H, W = 4, 16, 3, 64, 64
D = 256
NCORES = 8

F32 = mybir.dt.float32
BF16 = mybir.dt.bfloat16
F8 = mybir.dt.float8e4

WIN = 11          # feats/conv t-window frames per core
WP = WIN + 2      # padded window
M35 = [0, 1, 1, 1, 2]          # 5-grid pos -> 3-class variant
M57 = [0, 1, 2, 2, 2, 3, 4]    # 7-grid pos -> 5-grid src index
# expansion groups (dst0, dstlen, src0, srclen) along one axis for 5->7
G57 = [(0, 2, 0, 2), (2, 3, 2, 1), (5, 2, 3, 2)]
# h2-row groups for segmented stats reduction (h = 2*h2 + hpar)
H2G = [(0, 1), (1, 2), (2, 30), (30, 31), (31, 32)]
NG = len(H2G)
WCLS_BOUNDS = [0, 1, 2, 3, 61, 62, 63, 64]

NU1 = 3 * NG * WIN       # 165
NUC = NG * WIN           # 55
NSTAT = NU1 + NUC + 1    # 221


def _emit(nc, a_in, a_out):
    ctx = ExitStack()
    tc = tile.TileContext(nc)
    with tc, ctx:
        io = ctx.enter_context(tc.tile_pool(name="io", bufs=1))
        wkp = ctx.enter_context(tc.tile_pool(name="wkp", bufs=3))
        work = ctx.enter_context(tc.tile_pool(name="work", bufs=1))
        ps = ctx.enter_context(tc.tile_pool(name="ps", bufs=1, space="PSUM"))

        # ---------- early memsets (Pool; no deps) ----------
        fpad = work.tile([128, 2 * WP], BF16, tag="fpad")
        nc.gpsimd.memset(fpad[:], 0.0)
        h1p = work.tile([128, WP * 7 * 7], BF16, tag="h1p")
        nc.gpsimd.memset(h1p[:], 0.0)
        h2p = work.tile([65, 7 * WP * 9 + 2], BF16, tag="h2p")
        nc.gpsimd.memset(h2p[0:64, :], 0.0)
        nc.gpsimd.memset(h2p[64:65, :], 1.0)

        # ---------- input DMAs (serialized by the DMA engine) ----------
        cons = io.tile([128, 8], F32)
        nc.sync.dma_start(cons[:], a_in["consts"])
        om = io.tile([128, 1262], BF16)
        nc.sync.dma_start(om[:], a_in["obsmask"])
        obs = om[:, 0:1056]
        msk8 = io.tile([128, 704], mybir.dt.uint8)
        nc.sync.dma_start(msk8[:], a_in["msk"])
        kmbf = work.tile([128, 704], BF16, tag="kmbf")
        nc.vector.tensor_copy(kmbf[:], msk8[:])
        keep = kmbf[:, 0:352]
        mst = kmbf[:, 352:704]

        # ---------- PE warm-up (p-state ramp) ----------
        warm_ps = ps.tile([2, 512], F32, tag="warm")
        for i in range(24):
            nc.tensor.matmul(
                warm_ps[:, 0:8], cons[:, 0:2], cons[:, 0:8],
                start=(i == 0), stop=(i == 23),
            )
        # early act-table preload (off the critical path)
        junk = work.tile([2, 8], F32, tag="junk")
        nc.scalar.activation(
            junk[:], warm_ps[:, 0:8], mybir.ActivationFunctionType.Relu
        )
        for i in range(10):
            nc.tensor.matmul(
                warm_ps[:], om[:, 0:2], om[:, 0:512],
                start=(i == 0), stop=(i == 9),
            )

        # ---------- encoder input masking ----------
        xt = work.tile([128, 3 * 32 * WIN], BF16, tag="xt")
        for c in range(C):
            sl = slice(c * 32 * WIN, (c + 1) * 32 * WIN)
            nc.vector.tensor_mul(xt[:, sl], obs[:, sl], keep)

        # bias vectors on om row 0 (cols 1774:1902 b1, 1902:1966 b2);
        # ones row at partition 0 for psum bias preloads
        onesrow = work.tile([1, WIN * 25], BF16, tag="onesrow")
        nc.gpsimd.memset(onesrow[:], 1.0)

        # ---------- encoder matmuls: feats [d, tau] in two d-halves ----
        feats0 = ps.tile([128, WIN], F32, tag="feats0")
        feats1 = ps.tile([128, WIN], F32, tag="feats1")
        fps = [feats0, feats1]
        for g in range(4):
            wk = wkp.tile([128, 24 * D], F8, tag="wk")
            nc.sync.dma_start(wk[:], a_in["wencT"][g])
            for r in range(24):
                ki = g * 24 + r
                for u in range(2):
                    nc.tensor.matmul(
                        fps[u][:],
                        wk[:, r * D + u * 128: r * D + (u + 1) * 128],
                        xt[:, ki * WIN: (ki + 1) * WIN],
                        start=(ki == 0),
                        stop=(ki == 95),
                    )

        # conv weights after wencT (conv chain is later anyway); wc1 in two
        # pieces so conv1 phase A starts on the first
        wc1 = io.tile([128, 54 * 128], F8)
        wc2 = io.tile([128, 27 * 64], F8)
        wc3 = io.tile([128, 98], BF16)
        nc.sync.dma_start(wc1[:, 0:36 * 128], a_in["wc1"][:, 0:36 * 128])
        nc.sync.dma_start(wc2[:], a_in["wc2"])
        nc.sync.dma_start(wc1[:, 36 * 128:], a_in["wc1"][:, 36 * 128:])
        nc.sync.dma_start(wc3[:], a_in["wc3"])

        # feats + b_enc (per-partition broadcast add) -> fpad on DVE
        for u in range(2):
            nc.vector.tensor_add(
                fpad[:, u * WP + 1: u * WP + 1 + WIN], fps[u][:],
                cons[:, u: u + 1].broadcast_to([128, WIN]),
            )

        # ---------- mask stats (DVE; overlapped with DMA/encoder) ------
        mo = work.tile([128, 1056], BF16, tag="mo")
        for c in range(C):
            sl = slice(c * 32 * WIN, (c + 1) * 32 * WIN)
            nc.vector.tensor_mul(mo[:, sl], obs[:, sl], mst)
        mo2 = work.tile([128, 1056], BF16, tag="mo2")
        nc.vector.tensor_mul(mo2[:], mo[:], obs)
        smv = work.tile([128, NSTAT], BF16, tag="smv")
        vmo = mo[:].rearrange("p (c h t) -> p c t h", c=3, h=32, t=WIN)
        vms = mst.rearrange("p (h t) -> p t h", h=32, t=WIN)
        vU1 = smv[:, 0:NU1].rearrange("p (c g t) -> p c g t", c=3, g=NG, t=WIN)
        vUc = smv[:, NU1:NU1 + NUC].rearrange("p (g t) -> p g t", g=NG, t=WIN)
        with nc.allow_low_precision(reason="short class sums; bf16 ok"):
            for gi, (h0, h1_) in enumerate(H2G):
                nc.vector.reduce_sum(
                    vU1[:, :, gi, :], vmo[:, :, :, h0:h1_],
                    axis=mybir.AxisListType.X,
                )
                nc.vector.reduce_sum(
                    vUc[:, gi, :], vms[:, :, h0:h1_], axis=mybir.AxisListType.X
                )
            nc.vector.reduce_sum(
                smv[:, NSTAT - 1: NSTAT], mo2[:], axis=mybir.AxisListType.X
            )
        # class matmul: [14, NSTAT] = wclsT^T @ smv   (after encoder on PE)
        sps = ps.tile([14, NSTAT], F32, tag="stat")
        nc.tensor.matmul(sps[:], om[:, 1056:1070], smv[:], start=True, stop=True)
        outv = work.tile([14, NSTAT], F32, tag="outv")
        nc.scalar.activation(
            outv[:], sps[:], mybir.ActivationFunctionType.Identity
        )
        nc.sync.dma_start(a_out["outv"], outv[:])

        # ---------- conv1: direct 5x5 grid via W1eff variants ----------
        # phase A groups use variants in the first wc1 piece (v <= 4)
        groups = sorted(
            ((a5, b5) for a5 in range(5) for b5 in range(5)),
            key=lambda ab: (M35[ab[0]] * 3 + M35[ab[1]] > 5,),
        )
        c1 = ps.tile([128, WIN * 5 * 5], F32, tag="c1")
        vc1 = c1[:].rearrange("p (t a b) -> p t a b", t=WIN, a=5, b=5)
        vh1p = h1p[:].rearrange("p (t a b) -> p t a b", t=WP, a=7, b=7)
        for gi, (a5, b5) in enumerate(groups):
            v = M35[a5] * 3 + M35[b5]
            nc.tensor.matmul(
                vc1[:, :, a5, b5], om[0:1, 1070:1198], onesrow[:, 0:WIN],
                start=True, stop=False,
            )
            for kt in range(3):
                for u in range(2):
                    nc.tensor.matmul(
                        vc1[:, :, a5, b5],
                        wc1[:, ((v * 3 + kt) * 2 + u) * 128:
                            ((v * 3 + kt) * 2 + u + 1) * 128],
                        fpad[:, u * WP + kt: u * WP + kt + WIN],
                        start=False,
                        stop=(kt == 2 and u == 1),
                    )
            if gi == 19:
                # phase A (a5 0..3) complete: relu that region now so
                # conv2's kh=0 taps need not wait for phase B
                nc.vector.tensor_relu(
                    vh1p[:, 1:1 + WIN, 1:5, 1:6], vc1[:, :, 0:4, :]
                )
        # phase B region (a5 = 4)
        nc.vector.tensor_relu(
            vh1p[:, 1:1 + WIN, 5:6, 1:6], vc1[:, :, 4:5, :]
        )

        # ---------- conv2: 3x3x3 on the padded 5-grid (kh-outer) ----------
        c2 = ps.tile([64, WIN * 5 * 5], F32, tag="c2")
        nc.tensor.matmul(
            c2[:], om[0:1, 1198:1262], onesrow[:, 0:WIN * 25],
            start=True, stop=False,
        )
        for kh in range(3):
            for kt in range(3):
                for kw in range(3):
                    tap = (kt * 3 + kh) * 3 + kw
                    nc.tensor.matmul(
                        c2[:],
                        wc2[:, tap * 64: (tap + 1) * 64],
                        vh1p[:, kt:kt + WIN, kh:kh + 5, kw:kw + 5],
                        start=False,
                        stop=(kh == 2 and kt == 2 and kw == 2),
                    )
        # relu -> h2 (bf16) on DVE, then a-axis-only 5->7 expansion
        # (conv3 addresses the b-axis 5-grid directly via B5MAP)
        h2 = work.tile([64, WIN * 5 * 5], BF16, tag="h2")
        nc.vector.tensor_relu(h2[:], c2[:])
        # src dims permuted to (b5, t, a5) to match h2p5 layout
        vh2b = h2[:].rearrange("p (t a b) -> p b t a", t=WIN, a=5, b=5)
        vh2p = h2p[0:64, 0:7 * WP * 9].rearrange(
            "p (b t a) -> p b t a", b=7, t=WP, a=9)
        for ci, (da, la, sa, lsa) in enumerate(G57):
            src = vh2b[:, :, :, sa:sa + lsa]
            if lsa == 1:
                src = src.broadcast_to([64, 5, WIN, la])
            dst = vh2p[:, 1:6, 1:1 + WIN, 1 + da:1 + da + la]
            if ci == 0:
                nc.gpsimd.tensor_copy(dst, src)
            elif ci == 1:
                nc.vector.tensor_copy(dst, src)
            else:
                nc.scalar.copy(dst, src)

        # ---------- conv3: vox-stationary (flat 99-col slices; 2 junk
        # a-rows per tau that the host ignores), b3 folded via ones row --
        NVX = 9 * WIN  # 99
        B5MAP = [0, 1, 2, 3, 3, 3, 4, 5, 6]
        c3 = ps.tile([NVX, 21], F32, tag="c3")
        for bc in range(7):
            for kt in range(3):
                for kh in range(3):
                    for kw in range(3):
                        tap = (kt * 3 + kh) * 3 + kw
                        rows = 65 if tap == 0 else 64
                        base = B5MAP[kw + bc] * WP * 9 + kt * 9 + kh
                        nc.tensor.matmul(
                            c3[:, bc * 3: (bc + 1) * 3],
                            h2p[0:rows, base: base + NVX],
                            wc3[0:rows, tap * 3: (tap + 1) * 3],
                            start=(tap == 0),
                            stop=(tap == 26),
                        )
        recon = work.tile([NVX, 21], F32, tag="recon")
        nc.vector.tensor_copy(recon[:], c3[:])
        nc.sync.dma_start(a_out["recon"], recon[:])


_CACHE = {}


def _build():
    if "nc" in _CACHE:
        return _CACHE["nc"]
    nc = bacc.Bacc("TRN2", target_bir_lowering=False, debug=False)
    a_in = {}

    def din(name, shape, dt):
        a_in[name] = nc.dram_tensor(name, shape, dt, kind="ExternalInput").ap()

    din("consts", (128, 8), F32)
    din("obsmask", (128, 1262), BF16)
    din("msk", (128, 704), mybir.dt.uint8)
    din("wencT", (4, 128, 24 * D), F8)
    din("wc1", (128, 54 * 128), F8)
    din("wc2", (128, 27 * 64), F8)
    din("wc3", (128, 98), BF16)
    a_out = {}
    for name, shape in [("recon", (9 * WIN, 21)), ("outv", (14, NSTAT))]:
        a_out[name] = nc.dram_tensor(name, shape, F32, kind="ExternalOutput").ap()
    _emit(nc, a_in, a_out)
    nc.compile()
    _CACHE["nc"] = nc
    return nc


def make_in_maps(obs_strip, mask, W_enc, b_enc, w1, b1, w2, b2, w3, b3):
    import ml_dtypes

    bf16 = ml_dtypes.bfloat16
    f8 = ml_dtypes.float8_e4m3

    obs_strip = np.asarray(obs_strip, np.float32)
    mask_f = np.asarray(mask).astype(np.float32)

    # --- shared weights ---
    wencT = np.ascontiguousarray(
        np.asarray(W_enc, np.float32)
        .reshape(D, 3, 32, 2, 64)
        .transpose(3, 4, 1, 2, 0)
        .reshape(128, 96, D)
        .reshape(128, 4, 24 * D)
        .transpose(1, 0, 2)
    ).astype(f8)

    K = {0: [1, 2], 1: [0, 1, 2], 2: [0, 1]}
    w1 = np.asarray(w1, np.float32)
    W1e = np.zeros((9, 3, 128, 2, 128), np.float32)  # [v, kt, c, u, dmod]
    for va in range(3):
        for vb in range(3):
            for kt in range(3):
                eff = w1[:, :, kt][:, :, K[va]][:, :, :, K[vb]].sum((2, 3))
                W1e[va * 3 + vb, kt] = eff.reshape(128, 2, 128)
    wc1 = np.ascontiguousarray(
        W1e.transpose(4, 0, 1, 3, 2).reshape(128, 54 * 128)
    ).astype(f8)

    wc2 = np.ascontiguousarray(
        np.asarray(w2, np.float32).transpose(1, 2, 3, 4, 0).reshape(128, 27 * 64)
    ).astype(f8)

    wc3 = np.zeros((128, 98), np.float32)
    wc3[0:64, 0:81] = np.asarray(w3, np.float32).transpose(1, 2, 3, 4, 0).reshape(64, 81)
    wc3[64, 0:3] = np.asarray(b3, np.float32)
    wc3 = wc3.astype(bf16)
    wcls = np.zeros((128, 14), np.float32)
    for u in range(2):
        for j in range(7):
            w0, w1_ = WCLS_BOUNDS[j], WCLS_BOUNDS[j + 1]
            wcls[u * 64 + w0: u * 64 + w1_, u * 7 + j] = 1.0

    consts = np.zeros((128, 8), np.float32)
    consts[:, 0] = np.asarray(b_enc, np.float32)[0:128]
    consts[:, 1] = np.asarray(b_enc, np.float32)[128:256]
    consts[:, 2] = np.asarray(b1, np.float32)
    consts[0:64, 3] = np.asarray(b2, np.float32)

    shared = {"wencT": wencT, "wc1": wc1, "wc2": wc2, "wc3": wc3,
              "consts": consts}

    def perm_obs(o):  # [t, C, H, W] -> [128, (c, h2, t)]
        t = o.shape[0]
        return (o.reshape(t, 3, 32, 2, 64).transpose(3, 4, 1, 2, 0)
                .reshape(128, 3 * 32 * t))

    def perm_msk(m):  # [t, H, W] -> [128, (h2, t)]
        t = m.shape[0]
        return (m.reshape(t, 32, 2, 64).transpose(2, 3, 1, 0)
                .reshape(128, 32 * t))

    in_maps = []
    for core in range(NCORES):
        b, th = core // 2, core % 2
        s = 5 * th
        om = np.zeros((128, 1262), np.float32)
        om[:, 1056:1070] = wcls
        om[0, 1070:1198] = np.asarray(b1, np.float32)
        om[0, 1198:1262] = np.asarray(b2, np.float32)
        om[:, 0:1056] = perm_obs(obs_strip[b, s:s + WIN])
        msk = np.zeros((128, 704), np.uint8)
        msk[:, 0:352] = perm_msk(1.0 - mask_f[b, s:s + WIN])
        mstat = mask_f[b].copy()
        if th == 0:
            mstat[8:] = 0.0
        else:
            mstat[:8] = 0.0
        msk[:, 352:704] = perm_msk(mstat[s:s + WIN])
        in_maps.append({"obsmask": om.astype(bf16), "msk": msk, **shared})
    return in_maps


# host-side fold: (g, hpar) -> h class contributions
HCLS_SRC = [[(0, 0)], [(0, 1)], [(1, 0)],
            [(1, 1), (2, 0), (2, 1), (3, 0)],
            [(3, 1)], [(4, 0)], [(4, 1)]]


def assemble(results):
    total_sq = 0.0
    total_cnt = 0.0
    total_s2 = 0.0
    for core in range(NCORES):
        r = results[core]
        rec = r["recon"].astype(np.float64).reshape(WIN, 9, 7, 3)[:, 0:7]  # [tau,a,b,c]
        outv = r["outv"].astype(np.float64)
        U1 = outv[:, 0:NU1].reshape(2, 7, 3, NG, WIN)   # [u,j,c,g,tau]
        Uc = outv[:, NU1:NU1 + NUC].reshape(2, 7, NG, WIN)  # [u,j,g,tau]
        total_s2 += float(outv[:, NSTAT - 1].sum())
        s1 = np.zeros((3, 7, 7, WIN))   # [c, hcls, wcls, tau]
        cnt = np.zeros((7, 7, WIN))     # [hcls, wcls, tau]
        for i in range(7):
            for (g, u) in HCLS_SRC[i]:
                s1[:, i] += U1[u, :, :, g, :].transpose(1, 0, 2)
                cnt[i] += Uc[u, :, g, :]
        rt = rec.transpose(3, 1, 2, 0)  # [c, a(hcls), b(wcls), tau]
        total_sq += float((rt * rt * cnt[None]).sum() - 2.0 * (rt * s1).sum())
        total_cnt += float(cnt.sum())
    loss = (total_sq + total_s2) / max(total_cnt * C, 1.0)
    return np.float32(loss)


def kernel(**inputs):
    nc = _build()
    in_maps = make_in_maps(**inputs)
    res = bass_utils.run_bass_kernel_spmd(nc, in_maps, core_ids=list(range(NCORES)))
    _CACHE["last_res"] = res
    return assemble(res.results)


if __name__ == "__main__":
    pass


# revision 49
# speedup vs baseline: 3.2868x; 1.0462x over previous
"""Masked video loss kernel for TRN2 (8 NeuronCores, SPMD).

Algorithmic structure exploited:
- The decoder input feat_3d is spatially constant (broadcast of per-frame
  features over H=W=64), so conv1 collapses to a per-frame linear map with
  9 edge-variant weight sums (W1eff), evaluated directly on a 5x5 class
  grid. conv2 runs as a true 3x3x3 conv on the (padded) 5-grid; its output
  is expanded to the 7-grid on which conv3 produces the 7x7 recon classes.
  All exact (class algebra), not approximations.
- Masked MSE folds through per-class stats: sum (r-o)^2 = r^2 cnt - 2 r s1
  + s2 per (t, 7x7 class); s1/cnt come from one 0/1-matrix PE matmul plus
  segmented DVE reduces.

Sharding: core = 2*b + th. Each core handles batch b and an 11-frame
t-window starting at s = 5*th (host shifts the data, so the program is
SPMD-uniform); decoder outputs are valid for the core's 8-frame t-half,
and mask stats are host-zeroed outside that half.

Precision: W_enc / w1eff / w2 in fp8-e4m3 (stationary operands), obs /
activations bf16, accumulation fp32. Measured end-to-end loss rel err
~5e-3 (gate 2e-2).
"""

import sys

sys.path.insert(0, "/opt/trn_rl_repo")

from contextlib import ExitStack  # noqa: E402

import numpy as np  # noqa: E402

import concourse.bacc as bacc  # noqa: E402
import concourse.mybir as mybir  # noqa: E402
import concourse.tile as tile  # noqa: E402
from concourse import bass_utils  # noqa: E402

B, T, C, H, W = 4, 16, 3, 64, 64
D = 256
NCORES = 8

F32 = mybir.dt.float32
BF16 = mybir.dt.bfloat16
F8 = mybir.dt.float8e4

WIN = 11          # feats/conv t-window frames per core
WP = WIN + 2      # padded window
M35 = [0, 1, 1, 1, 2]          # 5-grid pos -> 3-class variant
M57 = [0, 1, 2, 2, 2, 3, 4]    # 7-grid pos -> 5-grid src index
# expansion groups (dst0, dstlen, src0, srclen) along one axis for 5->7
G57 = [(0, 2, 0, 2), (2, 3, 2, 1), (5, 2, 3, 2)]
# h2-row groups for segmented stats reduction (h = 2*h2 + hpar)
H2G = [(0, 1), (1, 2), (2, 30), (30, 31), (31, 32)]
NG = len(H2G)
WCLS_BOUNDS = [0, 1, 2, 3, 61, 62, 63, 64]

NU1 = 3 * NG * WIN       # 165
NUC = NG * WIN           # 55
NSTAT = NU1 + NUC + 1    # 221


def _emit(nc, a_in, a_out):
    ctx = ExitStack()
    tc = tile.TileContext(nc)
    with tc, ctx:
        io = ctx.enter_context(tc.tile_pool(name="io", bufs=1))
        wkp = ctx.enter_context(tc.tile_pool(name="wkp", bufs=3))
        work = ctx.enter_context(tc.tile_pool(name="work", bufs=1))
        ps = ctx.enter_context(tc.tile_pool(name="ps", bufs=1, space="PSUM"))

        # ---------- early memsets (Pool; no deps) ----------
        fpad = work.tile([128, 2 * WP], BF16, tag="fpad")
        nc.gpsimd.memset(fpad[:], 0.0)
        h1p = work.tile([128, WP * 7 * 7], BF16, tag="h1p")
        nc.gpsimd.memset(h1p[:], 0.0)
        h2p = work.tile([65, 7 * WP * 9 + 2], BF16, tag="h2p")
        nc.gpsimd.memset(h2p[0:64, :], 0.0)
        nc.gpsimd.memset(h2p[64:65, :], 1.0)

        # ---------- input DMAs (serialized by the DMA engine) ----------
        # one merged leading tensor: [bf16 consts block | fp8 obs | u8 masks]
        in0 = io.tile([128, 2176], mybir.dt.uint8)
        nc.sync.dma_start(in0[:], a_in["in0"])
        om = in0[:, 0:416].bitcast(BF16)   # [128, 208]
        obs8 = in0[:, 416:1472].bitcast(F8)
        msk8 = in0[:, 1472:2176]
        obsb = work.tile([128, 1056], BF16, tag="obsb")
        obs = obsb[:, 0:1056]
        kmbf = work.tile([128, 704], BF16, tag="kmbf")
        keep = kmbf[:, 0:352]
        mst = kmbf[:, 352:704]
        nc.vector.tensor_copy(keep, msk8[:, 0:352])  # u8 -> bf16

        # ---------- PE warm-up (p-state ramp) ----------
        warm_ps = ps.tile([2, 512], F32, tag="warm")
        for i in range(24):
            nc.tensor.matmul(
                warm_ps[:, 0:8], om[:, 0:2], om[:, 0:8],
                start=(i == 0), stop=(i == 23),
            )
        # early act-table preload (off the critical path)
        junk = work.tile([2, 8], F32, tag="junk")
        nc.scalar.activation(
            junk[:], warm_ps[:, 0:8], mybir.ActivationFunctionType.Relu
        )
        for i in range(10):
            nc.tensor.matmul(
                warm_ps[:, 0:128], om[:, 0:2], om[:, 0:128],
                start=(i == 0), stop=(i == 9),
            )

        # ---------- encoder input masking (cast + mask per c-block) ----
        xt = work.tile([128, 3 * 32 * WIN], BF16, tag="xt")
        for c in range(C):
            sl = slice(c * 32 * WIN, (c + 1) * 32 * WIN)
            nc.vector.tensor_copy(obsb[:, sl], obs8[:, sl])
            nc.vector.tensor_mul(xt[:, sl], obs[:, sl], keep)
        nc.vector.tensor_copy(mst, msk8[:, 352:704])

        # bias vectors on om row 0 (cols 1774:1902 b1, 1902:1966 b2);
        # ones row at partition 0 for psum bias preloads
        onesrow = work.tile([1, WIN * 25], BF16, tag="onesrow")
        nc.gpsimd.memset(onesrow[:], 1.0)

        # ---------- encoder matmuls: feats [d, tau] in two d-halves ----
        feats0 = ps.tile([128, WIN], F32, tag="feats0")
        feats1 = ps.tile([128, WIN], F32, tag="feats1")
        fps = [feats0, feats1]
        for g in range(4):
            wk = wkp.tile([128, 24 * D], F8, tag="wk")
            nc.sync.dma_start(wk[:], a_in["wencT"][g])
            for r in range(24):
                ki = g * 24 + r
                for u in range(2):
                    nc.tensor.matmul(
                        fps[u][:],
                        wk[:, r * D + u * 128: r * D + (u + 1) * 128],
                        xt[:, ki * WIN: (ki + 1) * WIN],
                        start=(ki == 0),
                        stop=(ki == 95),
                    )

        # conv weights after wencT (conv chain is later anyway); wc1 in two
        # pieces so conv1 phase A starts on the first
        wc1 = io.tile([128, 54 * 128], F8)
        wc23t = io.tile([128, 1924], mybir.dt.uint8)
        wc2 = wc23t[:, 0:1728].bitcast(F8)
        wc3 = wc23t[:, 1728:1924].bitcast(BF16)
        nc.sync.dma_start(wc1[:, 0:36 * 128], a_in["wc1"][:, 0:36 * 128])
        nc.sync.dma_start(wc1[:, 36 * 128:], a_in["wc1"][:, 36 * 128:])
        nc.sync.dma_start(wc23t[:], a_in["wc23"])

        # feats + b_enc (per-partition broadcast add) -> fpad on DVE
        for u in range(2):
            nc.vector.tensor_add(
                fpad[:, u * WP + 1: u * WP + 1 + WIN], fps[u][:],
                om[:, 206 + u: 207 + u].broadcast_to([128, WIN]),
            )

        # ---------- mask stats (DVE; overlapped with DMA/encoder) ------
        mo = work.tile([128, 1056], BF16, tag="mo")
        for c in range(C):
            sl = slice(c * 32 * WIN, (c + 1) * 32 * WIN)
            nc.vector.tensor_mul(mo[:, sl], obs[:, sl], mst)
        mo2 = work.tile([128, 1056], BF16, tag="mo2")
        nc.vector.tensor_mul(mo2[:], mo[:], obs)
        smv = work.tile([128, NSTAT], BF16, tag="smv")
        vmo = mo[:].rearrange("p (c h t) -> p c t h", c=3, h=32, t=WIN)
        vms = mst.rearrange("p (h t) -> p t h", h=32, t=WIN)
        vU1 = smv[:, 0:NU1].rearrange("p (c g t) -> p c g t", c=3, g=NG, t=WIN)
        vUc = smv[:, NU1:NU1 + NUC].rearrange("p (g t) -> p g t", g=NG, t=WIN)
        with nc.allow_low_precision(reason="short class sums; bf16 ok"):
            for gi, (h0, h1_) in enumerate(H2G):
                nc.vector.reduce_sum(
                    vU1[:, :, gi, :], vmo[:, :, :, h0:h1_],
                    axis=mybir.AxisListType.X,
                )
                nc.vector.reduce_sum(
                    vUc[:, gi, :], vms[:, :, h0:h1_], axis=mybir.AxisListType.X
                )
            nc.vector.reduce_sum(
                smv[:, NSTAT - 1: NSTAT], mo2[:], axis=mybir.AxisListType.X
            )
        # class matmul: [14, NSTAT] = wclsT^T @ smv   (after encoder on PE)
        sps = ps.tile([14, NSTAT], F32, tag="stat")
        nc.tensor.matmul(sps[:], om[:, 0:14], smv[:], start=True, stop=True)
        outv = work.tile([14, NSTAT], F32, tag="outv")
        nc.scalar.activation(
            outv[:], sps[:], mybir.ActivationFunctionType.Identity
        )
        nc.sync.dma_start(a_out["outv"], outv[:])

        # ---------- conv1: direct 5x5 grid via W1eff variants ----------
        # phase A groups use variants in the first wc1 piece (v <= 4)
        groups = sorted(
            ((a5, b5) for a5 in range(5) for b5 in range(5)),
            key=lambda ab: (M35[ab[0]] * 3 + M35[ab[1]] > 5,),
        )
        c1 = ps.tile([128, WIN * 5 * 5], F32, tag="c1")
        vc1 = c1[:].rearrange("p (t a b) -> p t a b", t=WIN, a=5, b=5)
        vh1p = h1p[:].rearrange("p (t a b) -> p t a b", t=WP, a=7, b=7)
        c2 = ps.tile([64, WIN * 5 * 5], F32, tag="c2")

        def c1_group(a5, b5):
            v = M35[a5] * 3 + M35[b5]
            nc.tensor.matmul(
                vc1[:, :, a5, b5], om[0:1, 14:142], onesrow[:, 0:WIN],
                start=True, stop=False,
            )
            for kt in range(3):
                for u in range(2):
                    nc.tensor.matmul(
                        vc1[:, :, a5, b5],
                        wc1[:, ((v * 3 + kt) * 2 + u) * 128:
                            ((v * 3 + kt) * 2 + u + 1) * 128],
                        fpad[:, u * WP + kt: u * WP + kt + WIN],
                        start=False,
                        stop=(kt == 2 and u == 1),
                    )

        def c2_taps(kh):
            for kt in range(3):
                for kw in range(3):
                    tap = (kt * 3 + kh) * 3 + kw
                    nc.tensor.matmul(
                        c2[:],
                        wc2[:, tap * 64: (tap + 1) * 64],
                        vh1p[:, kt:kt + WIN, kh:kh + 5, kw:kw + 5],
                        start=False,
                        stop=(kh == 2 and kt == 2 and kw == 2),
                    )

        # conv2 accumulation opens with the bias preload (no data deps)
        nc.tensor.matmul(
            c2[:], om[0:1, 142:206], onesrow[:, 0:WIN * 25],
            start=True, stop=False,
        )
        # conv1 phase A (a5 0..3; weights in wc1 piece 1)
        for a5, b5 in groups[:20]:
            c1_group(a5, b5)
        nc.vector.tensor_relu(
            vh1p[:, 1:1 + WIN, 1:5, 1:6], vc1[:, :, 0:4, :]
        )
        # conv1 phase B (a5 = 4; weights in wc1 piece 2)
        for a5, b5 in groups[20:]:
            c1_group(a5, b5)
        nc.vector.tensor_relu(
            vh1p[:, 1:1 + WIN, 5:6, 1:6], vc1[:, :, 4:5, :]
        )
        c2_taps(0)
        c2_taps(1)
        c2_taps(2)
        # relu -> h2 (bf16) on DVE, then a-axis-only 5->7 expansion
        # (conv3 addresses the b-axis 5-grid directly via B5MAP)
        h2 = work.tile([64, WIN * 5 * 5], BF16, tag="h2")
        nc.vector.tensor_relu(h2[:], c2[:])
        # src dims permuted to (b5, t, a5) to match h2p5 layout
        vh2b = h2[:].rearrange("p (t a b) -> p b t a", t=WIN, a=5, b=5)
        vh2p = h2p[0:64, 0:7 * WP * 9].rearrange(
            "p (b t a) -> p b t a", b=7, t=WP, a=9)
        for ci, (da, la, sa, lsa) in enumerate(G57):
            src = vh2b[:, :, :, sa:sa + lsa]
            if lsa == 1:
                src = src.broadcast_to([64, 5, WIN, la])
            dst = vh2p[:, 1:6, 1:1 + WIN, 1 + da:1 + da + la]
            if ci == 0:
                nc.gpsimd.tensor_copy(dst, src)
            elif ci == 1:
                nc.vector.tensor_copy(dst, src)
            else:
                nc.scalar.copy(dst, src)

        # ---------- conv3: vox-stationary (flat 99-col slices; 2 junk
        # a-rows per tau that the host ignores), b3 folded via ones row --
        NVX = 9 * WIN  # 99
        B5MAP = [0, 1, 2, 3, 3, 3, 4, 5, 6]
        c3 = ps.tile([NVX, 21], F32, tag="c3")
        for bc in range(7):
            for kt in range(3):
                for kh in range(3):
                    for kw in range(3):
                        tap = (kt * 3 + kh) * 3 + kw
                        rows = 65 if tap == 0 else 64
                        base = B5MAP[kw + bc] * WP * 9 + kt * 9 + kh
                        nc.tensor.matmul(
                            c3[:, bc * 3: (bc + 1) * 3],
                            h2p[0:rows, base: base + NVX],
                            wc3[0:rows, tap * 3: (tap + 1) * 3],
                            start=(tap == 0),
                            stop=(tap == 26),
                        )
        recon = work.tile([NVX, 21], F32, tag="recon")
        nc.vector.tensor_copy(recon[:], c3[:])
        nc.sync.dma_start(a_out["recon"], recon[:])


_CACHE = {}


def _build():
    if "nc" in _CACHE:
        return _CACHE["nc"]
    nc = bacc.Bacc("TRN2", target_bir_lowering=False, debug=False)
    a_in = {}

    def din(name, shape, dt):
        a_in[name] = nc.dram_tensor(name, shape, dt, kind="ExternalInput").ap()

    din("in0", (128, 2176), mybir.dt.uint8)
    din("wencT", (4, 128, 24 * D), F8)
    din("wc1", (128, 54 * 128), F8)
    din("wc23", (128, 1924), mybir.dt.uint8)
    a_out = {}
    for name, shape in [("recon", (9 * WIN, 21)), ("outv", (14, NSTAT))]:
        a_out[name] = nc.dram_tensor(name, shape, F32, kind="ExternalOutput").ap()
    _emit(nc, a_in, a_out)
    nc.compile()
    _CACHE["nc"] = nc
    return nc


def make_in_maps(obs_strip, mask, W_enc, b_enc, w1, b1, w2, b2, w3, b3):
    import ml_dtypes

    bf16 = ml_dtypes.bfloat16
    f8 = ml_dtypes.float8_e4m3

    obs_strip = np.asarray(obs_strip, np.float32)
    mask_f = np.asarray(mask).astype(np.float32)

    # --- shared weights ---
    wencT = np.ascontiguousarray(
        np.asarray(W_enc, np.float32)
        .reshape(D, 3, 32, 2, 64)
        .transpose(3, 4, 1, 2, 0)
        .reshape(128, 96, D)
        .reshape(128, 4, 24 * D)
        .transpose(1, 0, 2)
    ).astype(f8)

    K = {0: [1, 2], 1: [0, 1, 2], 2: [0, 1]}
    w1 = np.asarray(w1, np.float32)
    W1e = np.zeros((9, 3, 128, 2, 128), np.float32)  # [v, kt, c, u, dmod]
    for va in range(3):
        for vb in range(3):
            for kt in range(3):
                eff = w1[:, :, kt][:, :, K[va]][:, :, :, K[vb]].sum((2, 3))
                W1e[va * 3 + vb, kt] = eff.reshape(128, 2, 128)
    wc1 = np.ascontiguousarray(
        W1e.transpose(4, 0, 1, 3, 2).reshape(128, 54 * 128)
    ).astype(f8)

    wc2 = np.ascontiguousarray(
        np.asarray(w2, np.float32).transpose(1, 2, 3, 4, 0).reshape(128, 27 * 64)
    ).astype(f8)

    wc3 = np.zeros((128, 98), np.float32)
    wc3[0:64, 0:81] = np.asarray(w3, np.float32).transpose(1, 2, 3, 4, 0).reshape(64, 81)
    wc3[64, 0:3] = np.asarray(b3, np.float32)
    wc3 = wc3.astype(bf16)
    wc23 = np.zeros((128, 1924), np.uint8)
    wc23[:, 0:1728] = wc2.view(np.uint8)
    wc23[:, 1728:1924] = wc3.view(np.uint8)
    wcls = np.zeros((128, 14), np.float32)
    for u in range(2):
        for j in range(7):
            w0, w1_ = WCLS_BOUNDS[j], WCLS_BOUNDS[j + 1]
            wcls[u * 64 + w0: u * 64 + w1_, u * 7 + j] = 1.0

    om = np.zeros((128, 208), np.float32)
    om[:, 0:14] = wcls
    om[0, 14:142] = np.asarray(b1, np.float32)
    om[0, 142:206] = np.asarray(b2, np.float32)
    om[:, 206] = np.asarray(b_enc, np.float32)[0:128]
    om[:, 207] = np.asarray(b_enc, np.float32)[128:256]
    om_u8 = np.ascontiguousarray(om.astype(bf16)).view(np.uint8)

    shared = {"wencT": wencT, "wc1": wc1, "wc23": wc23}

    def perm_obs(o):  # [t, C, H, W] -> [128, (c, h2, t)]
        t = o.shape[0]
        return (o.reshape(t, 3, 32, 2, 64).transpose(3, 4, 1, 2, 0)
                .reshape(128, 3 * 32 * t))

    def perm_msk(m):  # [t, H, W] -> [128, (h2, t)]
        t = m.shape[0]
        return (m.reshape(t, 32, 2, 64).transpose(2, 3, 1, 0)
                .reshape(128, 32 * t))

    in_maps = []
    for core in range(NCORES):
        b, th = core // 2, core % 2
        s = 5 * th
        in0 = np.zeros((128, 2176), np.uint8)
        in0[:, 0:416] = om_u8
        in0[:, 416:1472] = perm_obs(obs_strip[b, s:s + WIN]).astype(f8).view(np.uint8)
        in0[:, 1472:1824] = perm_msk(1.0 - mask_f[b, s:s + WIN])
        mstat = mask_f[b].copy()
        if th == 0:
            mstat[8:] = 0.0
        else:
            mstat[:8] = 0.0
        in0[:, 1824:2176] = perm_msk(mstat[s:s + WIN])
        in_maps.append({"in0": in0, **shared})
    return in_maps


# host-side fold: (g, hpar) -> h class contributions
HCLS_SRC = [[(0, 0)], [(0, 1)], [(1, 0)],
            [(1, 1), (2, 0), (2, 1), (3, 0)],
            [(3, 1)], [(4, 0)], [(4, 1)]]


def assemble(results):
    total_sq = 0.0
    total_cnt = 0.0
    total_s2 = 0.0
    for core in range(NCORES):
        r = results[core]
        rec = r["recon"].astype(np.float64).reshape(WIN, 9, 7, 3)[:, 0:7]  # [tau,a,b,c]
        outv = r["outv"].astype(np.float64)
        U1 = outv[:, 0:NU1].reshape(2, 7, 3, NG, WIN)   # [u,j,c,g,tau]
        Uc = outv[:, NU1:NU1 + NUC].reshape(2, 7, NG, WIN)  # [u,j,g,tau]
        total_s2 += float(outv[:, NSTAT - 1].sum())
        s1 = np.zeros((3, 7, 7, WIN))   # [c, hcls, wcls, tau]
        cnt = np.zeros((7, 7, WIN))     # [hcls, wcls, tau]
        for i in range(7):
            for (g, u) in HCLS_SRC[i]:
                s1[:, i] += U1[u, :, :, g, :].transpose(1, 0, 2)
                cnt[i] += Uc[u, :, g, :]
        rt = rec.transpose(3, 1, 2, 0)  # [c, a(hcls), b(wcls), tau]
        total_sq += float((rt * rt * cnt[None]).sum() - 2.0 * (rt * s1).sum())
        total_cnt += float(cnt.sum())
    loss = (total_sq + total_s2) / max(total_cnt * C, 1.0)
    return np.float32(loss)


def kernel(**inputs):
    nc = _build()
    in_maps = make_in_maps(**inputs)
    res = bass_utils.run_bass_kernel_spmd(nc, in_maps, core_ids=list(range(NCORES)))
    _CACHE["last_res"] = res
    return assemble(res.results)


if __name__ == "__main__":
    pass
